# revision 24
# baseline (speedup 1.0000x reference)
"""Trainium2 Bass kernel for nn_AttentionSampling (sparse window attention block).

Sharding: 8 cores, data-parallel, 1024 windows (half a batch) per core; windows are
independent so there is no cross-core communication. q/k live in a transposed
[d, tokens] layout (host pre-transposes) so projections run weight-stationary;
v stays in natural [keys, d] layout so the banded attention aggregation can run
as PE matmuls against the masked score matrix.

Structure (per 128-window / 512-key block):
- k-proj (N=512 bf16 matmuls) -> scores computed directly TRANSPOSED
  ([keys, windows], 16 N=128 matmuls) -> DVE band-mask multiply produces the
  sparse weight matrix W [512 keys, 128 windows] in bf16.
- Attention output via aggregate-then-project: avT = v_nat.T-contracted with W
  (16 N=128 MMs per block); the wv projection + residual add + LN1 stats then
  run once per 512-token superblock at N=512.
- Startup: all engines sit in a fixed ~6.3us rendezvous preamble; DMA issue
  starts ~6.5us and the first 512KB lands ~10.5us. The first q/k projections
  are ki-OUTER over per-d-tile DMA chunks (4 accumulator banks borrowed across
  psum pools) so the PE starts on chunk 0 instead of waiting for whole tiles.
  Warmup matmuls (memset by DVE, which is past the preamble at ~5.8us) bridge
  the issue->land window and open the HAM clock gate.
- DMA issue follows global need order round-robined across the sync/scalar/
  gpsimd queues (aggregate ~330 GB/s; each dma_start costs ~0.75us of NX issue
  time, so scalar-engine issues are spread between its ACT work).
- ffn for superblock 0 is pulled INTO the attention loop (ffn1 at b==7) and the
  post-loop tail interleaves sb0's ffn2/LN2 into the LN1(1) scalar-chain
  latency so the PE never idles waiting on DVE/ACT.
- LN broadcasts are copied PSUM->SBUF 16-bit by ACT so the DVE apply passes run
  at 2x rate; LN2 apply + output DMA are chunked per d-tile across two DMA
  queues to shrink the serial tail.
"""

import sys
import types

try:
    import antenv.axon_hooks  # noqa: F401
except ImportError:
    _m = types.ModuleType("antenv.axon_hooks")
    _m.get_axon_ntff_profile_hook = lambda: None
    _m.set_axon_ntff_profile_hook = lambda h: None
    sys.modules["antenv.axon_hooks"] = _m
    try:
        import antenv

        antenv.axon_hooks = _m
    except ImportError:
        pass

import contextlib

import numpy as np

import concourse.bass as bass
import concourse.bacc as bacc_mod
import concourse.mybir as mybir
import concourse.tile as tile
from concourse.bass import ts, ds
from concourse.bass_utils import run_bass_kernel_spmd

FP32 = mybir.dt.float32
FP16 = mybir.dt.float16
FP8 = mybir.dt.float8e4
AF = mybir.ActivationFunctionType
OP = mybir.AluOpType
DR = mybir.MatmulPerfMode.DoubleRow

MM_DT = mybir.dt.bfloat16  # non-fp8 matmul operands
# q/k and their projection weights are fp8 e4m3 (DoubleRow double-pumped
# matmuls, ~1.4x PE rate, half the DMA bytes). The projection weights are
# scaled x8 host-side so their small values avoid e4m3 subnormals; the k-side
# compensation folds into the band mask (1/8) + 8*b_k, the q-side into the
# drain scale. v/wv/ffn matmuls stay bf16 (fp8 there costs ~4x more error;
# measured final rel err ~1.5e-2 vs the 2e-2 gate). PSUM accumulation is fp32.
# The residual stream and LN stats run in bf16; rstd and apply scratch fp16.
WSCALE = 8.0

B, SQ, SK, D, F = 4, 2048, 8192, 512, 4
NCORES = 8
WPC = B * SQ // NCORES        # 1024 windows (= tokens) per core
KPC = WPC * F                 # 4096 keys per core
NBLK = WPC // 128             # 8 attention blocks: 128 windows / 512 keys
NSB = WPC // 512              # 2 superblocks of 512 tokens
DT = D // 128                 # 4 d-tiles
KC = 4                        # key chunks per block (512 keys / 128)
EPS = 1e-5
N_WARMUP = 10                 # PE warmup matmuls bridging DMA issue->land

_CACHE = {}


def build_program(use_vbias=True, affine1=True, affine2=True, use_qbias=True):
    nc = bacc_mod.Bacc(None, target_bir_lowering=False)

    qT_d = nc.dram_tensor("qT", [D, WPC], FP8, kind="ExternalInput")
    kT_d = nc.dram_tensor("kT", [D, KPC], FP8, kind="ExternalInput")
    vN_d = nc.dram_tensor("vN", [KPC, D], MM_DT, kind="ExternalInput")
    wq_d = nc.dram_tensor("w_q", [D, D], FP8, kind="ExternalInput")
    wk_d = nc.dram_tensor("w_k", [D, D], FP8, kind="ExternalInput")
    wv_d = nc.dram_tensor("w_v", [D, D], MM_DT, kind="ExternalInput")
    w1_d = nc.dram_tensor("ffn_w1", [D, D], MM_DT, kind="ExternalInput")
    w2_d = nc.dram_tensor("ffn_w2", [D, D], MM_DT, kind="ExternalInput")
    # All [D] bias/gain vectors are packed host-side into one [128, 8*DT]
    # tensor (order: b_q, b_k, ffn_b1, ffn_b2, ln1_g, ln1_b, ln2_g, ln2_b).
    consts_d = nc.dram_tensor("constsP", [128, 8 * DT], FP32, kind="ExternalInput")
    bvr_d = nc.dram_tensor("bv_row", [1, D], MM_DT, kind="ExternalInput")
    maskT_d = nc.dram_tensor("maskT", [128, KC, 128], MM_DT, kind="ExternalInput")
    outT_d = nc.dram_tensor("outT", [D, WPC], FP32, kind="ExternalOutput")

    qT_t = qT_d.rearrange("(o p) n -> p o n", p=128)
    kT_t = kT_d.rearrange("(o p) n -> p o n", p=128)
    vN_t = vN_d.rearrange("(nb kc p) d -> p nb kc d", p=128, kc=KC)
    outT_t = outT_d.rearrange("(o p) n -> p o n", p=128)

    with tile.TileContext(nc) as tc, contextlib.ExitStack() as ctx:
        # PSUM budget is 8 banks x 2KB: proj(2) + sc(1) + av(1) + ao(1|2) +
        # stats/bc shared tag(2) [+ srow(1) on the biased path] = 8.
        singles = ctx.enter_context(tc.tile_pool(name="singles", bufs=1))
        qin_p = ctx.enter_context(tc.tile_pool(name="qin", bufs=2))
        kin_p = ctx.enter_context(tc.tile_pool(name="kin", bufs=6))
        vin_p = ctx.enter_context(tc.tile_pool(name="vin", bufs=5))
        ktp_p = ctx.enter_context(tc.tile_pool(name="ktp", bufs=2))
        w_p = ctx.enter_context(tc.tile_pool(name="wsb", bufs=2))
        av_p = ctx.enter_context(tc.tile_pool(name="avsb", bufs=2))
        resid_p = ctx.enter_context(tc.tile_pool(name="resid", bufs=2))
        hT_p = ctx.enter_context(tc.tile_pool(name="hT", bufs=2))
        out_p = ctx.enter_context(tc.tile_pool(name="outp", bufs=2))
        small = ctx.enter_context(tc.tile_pool(name="small", bufs=1))
        ps_proj = ctx.enter_context(tc.tile_pool(name="ps_proj", bufs=2, space="PSUM"))
        ps_sc = ctx.enter_context(tc.tile_pool(name="ps_sc", bufs=1, space="PSUM"))
        ps_av = ctx.enter_context(tc.tile_pool(name="ps_av", bufs=1, space="PSUM"))
        ps_ao = ctx.enter_context(
            tc.tile_pool(name="ps_ao", bufs=1 if use_vbias else 2, space="PSUM"))
        ps_misc = ctx.enter_context(tc.tile_pool(name="ps_misc", bufs=2, space="PSUM"))

        consts_sb = singles.tile([128, 8 * DT], FP32, tag="constsP")
        _CONST_IDX = {"b_q": 0, "b_k": 1, "ffn_b1": 2, "ffn_b2": 3,
                      "ln1_g": 4, "ln1_b": 5, "ln2_g": 6, "ln2_b": 7}

        def load_b(name):
            i = _CONST_IDX[name]
            return consts_sb[:, i * DT : (i + 1) * DT]

        # Warmup scratch: memset on DVE (past the preamble ~5.8us; gpsimd's
        # SWDGE path must not be blocked behind memsets either way).
        warm_sb = singles.tile([128, 512], MM_DT, tag="warm")
        nc.vector.memset(warm_sb, 0.001)
        ones_colb = singles.tile([128, 1], MM_DT, tag="ones_colb")
        nc.vector.memset(ones_colb, 1.0)
        ones_rowb = singles.tile([1, 128], MM_DT, tag="ones_rowb")
        nc.vector.memset(ones_rowb, 1.0)
        ones_rowh = singles.tile([1, 128], FP16, tag="ones_rowh")
        nc.vector.memset(ones_rowh, 1.0)

        # ---- DMA issue: global need order, round-robined across queues ----
        # scalar(A) also runs ACT drains from ~15.5us, so its late loads are
        # spread into the loop; sync(S) has no compute -> all upfront.
        k_tiles = {}
        v_tiles = {}

        def prefetch_k(b, eng):
            k_t = kin_p.tile([128, DT, 512], FP8, tag="k_in", name="k_in")
            eng.dma_start(out=k_t, in_=kT_t[:, :, ts(b, 512)])
            k_tiles[b] = k_t

        def prefetch_v(b, eng):
            v_t = vin_p.tile([128, KC, 512], MM_DT, tag="v_in", name="v_in")
            eng.dma_start(out=v_t, in_=vN_t[:, b, :, :])
            v_tiles[b] = v_t

        def load_w(d, tg, eng, chunks=1, dt=MM_DT):
            t = singles.tile([128, DT, 512], dt, tag=tg)
            src = d.rearrange("(o p) n -> p o n", p=128)
            step = DT // chunks
            for c in range(chunks):
                eng.dma_start(out=t[:, c * step : (c + 1) * step, :],
                              in_=src[:, c * step : (c + 1) * step, :])
            return t

        # First-needed bytes are spread so no queue carries two of
        # {wq+q0, wk+k0} before the PE needs them:
        # A(scalar): consts, wq c0,c1, k2, k3, q1, v2  (+in-loop later)
        # S(sync):   q0 c0,c1, wk c0,c1, v0, k4, wv, k6, v5, w2, v7
        # G(gpsimd): k0, k1, maskT, v1, k5  (+in-loop later)
        nc.scalar.dma_start(out=consts_sb, in_=consts_d[:, :])
        wq_sb = load_w(wq_d, "wq", nc.scalar, chunks=2, dt=FP8)

        q_in = []
        for sb in range(NSB):
            t = qin_p.tile([128, DT, 512], FP8, tag="q_in", name="q_in")
            q_in.append(t)
        for c in range(2):
            nc.sync.dma_start(out=q_in[0][:, 2 * c : 2 * c + 2, :],
                              in_=qT_t[:, 2 * c : 2 * c + 2, ts(0, 512)])

        prefetch_k(0, nc.gpsimd)
        wk_sb = load_w(wk_d, "wk", nc.sync, chunks=2, dt=FP8)
        prefetch_k(1, nc.gpsimd)
        maskT = singles.tile([128, KC, 128], MM_DT, tag="maskT")
        nc.gpsimd.dma_start(out=maskT, in_=maskT_d[:, :, :])
        prefetch_k(2, nc.scalar)
        prefetch_v(0, nc.sync)
        prefetch_v(1, nc.gpsimd)
        prefetch_k(3, nc.scalar)
        prefetch_k(4, nc.sync)
        nc.scalar.dma_start(out=q_in[1], in_=qT_t[:, :, ts(1, 512)])
        prefetch_v(2, nc.scalar)
        prefetch_k(5, nc.gpsimd)
        bq_sb = load_b("b_q")
        bk_sb = load_b("b_k")
        bv_row = None
        if use_vbias:
            bv_row = singles.tile([1, 512], MM_DT, tag="bv_row")
            nc.scalar.dma_start(out=bv_row, in_=bvr_d[:, :])

        late = {}

        # ---- PE warmup: trip the HAM clock gate while DMAs land ----
        for i in range(N_WARMUP):
            wps = ps_proj.tile([128, 512], FP32, tag="proj_ps", name="warm_ps")
            nc.tensor.matmul(wps, lhsT=warm_sb[:, :128], rhs=warm_sb,
                             start=True, stop=True)

        qTp = singles.tile([128, DT, WPC], MM_DT, tag="qTp")
        xT = singles.tile([128, DT, WPC], MM_DT, tag="xT")

        def proj_drain(ps, bias_sb, out_ap, do, relu_dve, scale=1.0):
            if relu_dve:
                if scale == 1.0:
                    nc.vector.tensor_scalar(
                        out=out_ap, in0=ps,
                        scalar1=bias_sb[:, do : do + 1], scalar2=0.0,
                        op0=OP.add, op1=OP.max,
                    )
                else:
                    # bias known zero: fold the fp8 weight prescale out
                    nc.vector.tensor_scalar(
                        out=out_ap, in0=ps, scalar1=scale, scalar2=0.0,
                        op0=OP.mult, op1=OP.max,
                    )
            else:
                nc.scalar.activation(
                    out=out_ap, in_=ps, func=AF.Relu,
                    bias=bias_sb[:, do : do + 1], scale=scale,
                )

        def mm_proj(ps, w_sb, in_sb, do, n, dr):
            if dr:
                for c in range(2):
                    nc.tensor.matmul(
                        ps, lhsT=w_sb[:, 2 * c : 2 * c + 2, ts(do, 128)],
                        rhs=in_sb[:, 2 * c : 2 * c + 2, :n],
                        start=(c == 0), stop=(c == 1), perf_mode=DR,
                    )
            else:
                for ki in range(DT):
                    nc.tensor.matmul(
                        ps, lhsT=w_sb[:, ki, ts(do, 128)], rhs=in_sb[:, ki, :n],
                        start=(ki == 0), stop=(ki == DT - 1),
                    )

        def proj_T(w_sb, bias_sb, in_sb, out_sb, out_col0, n, relu_dve=False,
                   dr=False, scale=1.0):
            for do in range(DT):
                ps = ps_proj.tile([128, 512], FP32, tag="proj_ps", name="proj_ps")
                ps = ps[:, :n]
                mm_proj(ps, w_sb, in_sb, do, n, dr)
                proj_drain(ps, bias_sb, out_sb[:, do, ds(out_col0, n)], do,
                           relu_dve, scale)

        def proj_T_kiouter(w_sb, bias_sb, in_sb, out_sb, out_col0,
                           relu_dve=False, dr=False, scale=1.0):
            # contraction-OUTER accumulation across 4 borrowed psum banks so
            # each MM group depends only on the d-chunk of w/in that has landed.
            pss = [
                ps_proj.tile([128, 512], FP32, tag="proj_ps", name="kio_ps"),
                ps_proj.tile([128, 512], FP32, tag="proj_ps", name="kio_ps"),
                ps_ao.tile([128, 512], FP32, tag="ao_ps", name="kio_ps2"),
                (ps_sc if use_vbias else ps_ao).tile(
                    [128, 512], FP32,
                    tag="sc_ps" if use_vbias else "ao_ps", name="kio_ps3"),
            ]
            if dr:
                for c in range(2):
                    for do in range(DT):
                        nc.tensor.matmul(
                            pss[do], lhsT=w_sb[:, 2 * c : 2 * c + 2, ts(do, 128)],
                            rhs=in_sb[:, 2 * c : 2 * c + 2, :],
                            start=(c == 0), stop=(c == 1), perf_mode=DR,
                        )
            else:
                for ki in range(DT):
                    for do in range(DT):
                        nc.tensor.matmul(
                            pss[do], lhsT=w_sb[:, ki, ts(do, 128)], rhs=in_sb[:, ki, :],
                            start=(ki == 0), stop=(ki == DT - 1),
                        )
            for do in range(DT):
                proj_drain(pss[do], bias_sb, out_sb[:, do, ds(out_col0, 512)],
                           do, relu_dve, scale)

        # ---- phase 1: q projection superblock 0 (chunk-pipelined; sb1 is
        # deferred to b==3 since scores only need it from block 4) ----
        # qTp = relu(q @ (8*wq) / 8 + bq): DVE drain folds the 1/8 when bq==0,
        # else the ACT drain does scale=1/8 + bias.
        proj_T_kiouter(wq_sb, bq_sb, q_in[0], qTp, 0,
                       relu_dve=not use_qbias, dr=True, scale=1.0 / WSCALE)

        # ---- phase 2: attention, software-pipelined ----
        residT = {}  # superblock -> tile [128, DT, 512]
        kTp = {}     # block -> k-projection tile
        W_sb = {}    # block -> masked scoresT (the banded weight matrix)
        av4 = {}     # superblock -> [128, DT, 512] aggregated v (4 blocks)
        sr4 = {}     # superblock -> [1, 512] colsums of W (4 blocks)

        def emit_kproj(b, kiouter=False):
            # kTp8 = relu(k @ (8*wk) + 8*bk) = 8*kTp; the x8 cancels against
            # the 1/8-valued band mask in the score path.
            k_t = k_tiles.pop(b)
            kp = ktp_p.tile([128, DT, 512], MM_DT, tag="kTp", name="kTp")
            if kiouter:
                proj_T_kiouter(wk_sb, bk_sb, k_t, kp, 0, dr=True)
            else:
                proj_T(wk_sb, bk_sb, k_t, kp, 0, 512, dr=True)
            kTp[b] = kp

        def emit_scores(b):
            # scT[k, w] = sum_d kTp[d, k] * qTp[d, w] for this block's keys
            sc_ps = ps_sc.tile([128, KC, 128], FP32, tag="sc_ps", name="sc_ps")
            for kc in range(KC):
                for ki in range(DT):
                    nc.tensor.matmul(
                        sc_ps[:, kc, :],
                        lhsT=kTp[b][:, ki, ts(kc, 128)],
                        rhs=qTp[:, ki, ts(b, 128)],
                        start=(ki == 0), stop=(ki == DT - 1),
                    )
            del kTp[b]
            # band mask -> sparse weight matrix W (bf16, zero off-band)
            w_t = w_p.tile([128, KC, 128], MM_DT, tag="W", name="W")
            nc.vector.tensor_tensor(w_t[:], sc_ps[:], maskT[:], op=OP.mult)
            W_sb[b] = w_t

        def emit_vagg(b):
            sb, col = b // 4, (b % 4) * 128
            v_t = v_tiles.pop(b)
            w_t = W_sb[b]
            av_ps = ps_av.tile([128, DT, 128], FP32, tag="av_ps", name="av_ps")
            for dc in range(DT):
                for kc in range(KC):
                    nc.tensor.matmul(
                        av_ps[:, dc, :],
                        lhsT=v_t[:, kc, ts(dc, 128)],
                        rhs=w_t[:, kc, :],
                        start=(kc == 0), stop=(kc == KC - 1),
                    )
            if use_vbias:
                # srow[w] = sum_k W[k, w]  (for the bias term)
                sr_ps = ps_misc.tile([1, 128], FP32, tag="sr_ps", name="sr_ps", bufs=1)
                for kc in range(KC):
                    nc.tensor.matmul(
                        sr_ps, lhsT=ones_colb, rhs=w_t[:, kc, :],
                        start=(kc == 0), stop=(kc == KC - 1),
                    )
            if col == 0:
                av4[sb] = av_p.tile([128, DT, 512], MM_DT, tag="av4", name="av4")
                if use_vbias:
                    sr4[sb] = small.tile([1, 512], MM_DT, tag="sr4", name="sr4", bufs=2)
            nc.scalar.activation(
                out=av4[sb][:, :, ds(col, 128)], in_=av_ps[:], func=AF.Copy, scale=1.0)
            if use_vbias:
                nc.scalar.activation(
                    out=sr4[sb][:, ds(col, 128)], in_=sr_ps, func=AF.Copy, scale=1.0)
            del W_sb[b]

        def stats_pair():
            """PSUM accumulators for the LN token sums + the squares tile."""
            S1 = ps_misc.tile([1, 512], FP32, tag="st", name="st_sum")
            S2 = ps_misc.tile([1, 512], FP32, tag="st", name="st_sq")
            sqt = hT_p.tile([128, DT, 512], MM_DT, tag="sq", name="sq")
            return S1, S2, sqt

        def emit_stats_dt(st, resid_t, dt):
            """LN stats for one d-tile; interleaved into the producer stream
            one chunk behind so the PE never waits on the DVE square."""
            S1, S2, sqt = st
            nc.vector.tensor_tensor(
                sqt[:, dt, :], resid_t[:, dt, :], resid_t[:, dt, :], op=OP.mult)
            nc.tensor.matmul(S1, lhsT=ones_colb, rhs=resid_t[:, dt, :],
                             start=(dt == 0), stop=(dt == DT - 1))
            nc.tensor.matmul(S2, lhsT=ones_colb, rhs=sqt[:, dt, :],
                             start=(dt == 0), stop=(dt == DT - 1))

        def ln_finish_a(st):
            """LN scalar chain: token sums -> mean (bf16) and rstd (fp16).
            Serial path is 2 DVE ops + 1 ACT Rsqrt (the [1,512] single-lane
            ops cost ~700ns each, so op count dominates the tail):
            mean2 = S1^2/D (direct from PSUM, parallel with the mean copy);
            varD = S2 + D*eps - mean2; rstd = rsqrt(varD/D) = sqrt(D/varD)."""
            S1, S2, _ = st
            mean = small.tile([1, 512], MM_DT, tag="mean", name="mean", bufs=2)
            nc.scalar.activation(out=mean, in_=S1, func=AF.Copy, scale=1.0 / D)
            mean2 = small.tile([1, 512], FP32, tag="m2d", name="mean2")
            nc.vector.scalar_tensor_tensor(
                out=mean2, in0=mean, scalar=float(D), in1=mean,
                op0=OP.mult, op1=OP.mult,
            )
            varD = small.tile([1, 512], FP32, tag="varD", name="varD")
            nc.vector.scalar_tensor_tensor(
                out=varD, in0=S2, scalar=float(D) * EPS, in1=mean2,
                op0=OP.add, op1=OP.subtract,
            )
            r0 = small.tile([1, 512], FP32, tag="r0", name="r0")
            nc.vector.reciprocal_approx_fast(out=r0, in_=varD)
            rstd = small.tile([1, 512], FP16, tag="rstd", name="rstd", bufs=2)
            nc.scalar.activation(out=rstd, in_=r0, func=AF.Sqrt, scale=float(D))
            return mean, rstd

        def emit_ln_finish(st, resid_t, g_sb, gb_sb, out_cb,
                           out_dt_chunked=None, affine=True, mr=None,
                           last=False):
            """Broadcast + apply (+ output) given accumulated token sums."""
            if mr is None:
                mr = ln_finish_a(st)
            mean, rstd = mr

            # bc tiles share the "st" tag/banks; ACT copies them to 16-bit
            # SBUF immediately (frees the banks, enables DVE 2x applies, and
            # lets GpSimd - which has no PSUM port - run half the passes).
            bcm = ps_misc.tile([128, 512], FP32, tag="st", name="bcm")
            nc.tensor.matmul(bcm, lhsT=ones_rowb, rhs=mean, start=True, stop=True)
            bcr = ps_misc.tile([128, 512], FP32, tag="st", name="bcr")
            nc.tensor.matmul(bcr, lhsT=ones_rowh, rhs=rstd, start=True, stop=True)
            bcm_s = small.tile([128, 512], MM_DT, tag="bcm_s", name="bcm_s", bufs=2)
            nc.scalar.activation(out=bcm_s, in_=bcm, func=AF.Copy, scale=1.0)
            bcr_s = small.tile([128, 512], FP16, tag="bcr_s", name="bcr_s", bufs=2)
            nc.scalar.activation(out=bcr_s, in_=bcr, func=AF.Copy, scale=1.0)

            # Subs first (they only need bcm_s); the per-dt passes alternate
            # DVE / GpSimd so the two run concurrently. On the final finish
            # (last=True) the DVE mults read bcr straight from PSUM - no wait
            # on the bcr_s copy, and nothing later needs the bank.
            eng = (nc.vector, nc.gpsimd, nc.vector, nc.gpsimd)
            tmp = hT_p.tile([128, DT, 512], FP16, tag="tscr", name="tscr")
            for dt in range(DT):
                eng[dt].tensor_tensor(tmp[:, dt, :], resid_t[:, dt, :], bcm_s, op=OP.subtract)
            for dt in range(DT):
                if affine:
                    nc.vector.tensor_tensor(tmp[:, dt, :], tmp[:, dt, :], bcr_s, op=OP.mult)
                    nc.scalar.activation(
                        out=out_cb(dt), in_=tmp[:, dt, :], func=AF.Identity,
                        bias=gb_sb[:, dt : dt + 1], scale=g_sb[:, dt : dt + 1],
                    )
                else:
                    r_src = bcr if (last and eng[dt] is nc.vector) else bcr_s
                    eng[dt].tensor_tensor(out_cb(dt), tmp[:, dt, :], r_src, op=OP.mult)
                if out_dt_chunked:
                    out_dt_chunked(dt)

        def emit_aoproj_sb(sb):
            # ao projection for a whole superblock at N=512, with the residual
            # add and LN1 stats pipelined into the do-chunk stream.
            residT[sb] = resid_p.tile([128, DT, 512], MM_DT, tag="residT", name="residT")
            st = stats_pair()
            for do in range(DT):
                ao_ps = ps_ao.tile([128, 512], FP32, tag="ao_ps", name="ao_ps")
                for ki in range(DT):
                    nc.tensor.matmul(
                        ao_ps, lhsT=late["wv"][:, ki, ts(do, 128)],
                        rhs=av4[sb][:, ki, :],
                        start=(ki == 0), stop=(ki == DT - 1) and not use_vbias,
                    )
                if use_vbias:
                    nc.tensor.matmul(
                        ao_ps, lhsT=bv_row[:, ts(do, 128)], rhs=sr4[sb],
                        start=False, stop=True,
                    )
                nc.vector.tensor_tensor(
                    residT[sb][:, do, :], ao_ps, qTp[:, do, ts(sb, 512)], op=OP.add,
                )
                if do >= 1:
                    emit_stats_dt(st, residT[sb], do - 1)
            emit_stats_dt(st, residT[sb], DT - 1)
            return st

        def emit_ln1_finish(sb, st, mr=None):
            emit_ln_finish(st, residT[sb], late["g1"], late["gb1"],
                           lambda dt: xT[:, dt, ts(sb, 512)], affine=affine1,
                           mr=mr)

        def emit_ffn1(sb):
            hT = hT_p.tile([128, DT, 512], MM_DT, tag="hT", name="hT")
            for ht in range(DT):
                ps = ps_proj.tile([128, 512], FP32, tag="proj_ps", name="ffn1_ps")
                for ki in range(DT):
                    nc.tensor.matmul(
                        ps, lhsT=late["w1"][:, ki, ts(ht, 128)], rhs=xT[:, ki, ts(sb, 512)],
                        start=(ki == 0), stop=(ki == DT - 1),
                    )
                nc.scalar.activation(
                    out=hT[:, ht, :], in_=ps, func=AF.Relu,
                    bias=late["b1"][:, ht : ht + 1], scale=1.0,
                )
            return hT

        def emit_ffn2(sb, hT):
            resid2 = resid_p.tile([128, DT, 512], MM_DT, tag="resid2", name="resid2")
            st = stats_pair()
            for dt in range(DT):
                ps = ps_proj.tile([128, 512], FP32, tag="proj_ps", name="ffn2_ps")
                for hi in range(DT):
                    nc.tensor.matmul(
                        ps, lhsT=late["w2"][:, hi, ts(dt, 128)], rhs=hT[:, hi, :],
                        start=(hi == 0), stop=(hi == DT - 1),
                    )
                nc.vector.scalar_tensor_tensor(
                    out=resid2[:, dt, :], in0=ps, scalar=late["b2"][:, dt : dt + 1],
                    in1=xT[:, dt, ts(sb, 512)], op0=OP.add, op1=OP.add,
                )
                if dt >= 1:
                    emit_stats_dt(st, resid2, dt - 1)
            emit_stats_dt(st, resid2, DT - 1)
            return resid2, st

        def emit_ln2_finish(sb, resid2, st, mr=None, last=False):
            out_sb = out_p.tile([128, DT, 512], FP32, tag="out_sb", name="out_sb")

            def dma_dt(dt):
                (nc.sync if dt % 2 == 0 else nc.scalar).dma_start(
                    out=outT_t[:, dt, ts(sb, 512)], in_=out_sb[:, dt, :]
                )

            emit_ln_finish(st, resid2, late["g2"], late["gb2"],
                           lambda dt: out_sb[:, dt, :], out_dt_chunked=dma_dt,
                           affine=affine2, mr=mr, last=last)

        # pipeline: k-proj(b), scoresT(b-1), v-agg(b-2); ao projection / LN1 /
        # ffn for superblock 0 are interleaved into blocks 5-7.
        hT0 = None
        for b in range(NBLK + 2):
            if b < NBLK:
                emit_kproj(b, kiouter=(b == 0))
            if b == 0:
                late["wv"] = load_w(wv_d, "wv", nc.sync)
                late["g1"] = load_b("ln1_g")
                late["gb1"] = load_b("ln1_b")
                late["b1"] = load_b("ffn_b1")
                late["b2"] = load_b("ffn_b2")
                late["g2"] = load_b("ln2_g")
                late["gb2"] = load_b("ln2_b")
            elif b == 1:
                prefetch_v(3, nc.gpsimd)
                prefetch_k(6, nc.sync)
                prefetch_v(4, nc.scalar)
            elif b == 2:
                prefetch_k(7, nc.scalar)
                late["w1"] = load_w(w1_d, "w1", nc.gpsimd)
            elif b == 3:
                prefetch_v(5, nc.sync)
                late["w2"] = load_w(w2_d, "w2", nc.sync)
            elif b == 4:
                prefetch_v(6, nc.gpsimd)
                prefetch_v(7, nc.sync)
            if 1 <= b <= NBLK:
                emit_scores(b - 1)
            if 2 <= b <= NBLK + 1:
                emit_vagg(b - 2)
            if b == 3:
                # qproj sb1 after vagg(1): its q1/consts DMAs have landed by
                # now and scores(4) (at b==5) is the first consumer.
                proj_T(wq_sb, bq_sb, q_in[1], qTp, 512, 512,
                       relu_dve=not use_qbias, dr=True, scale=1.0 / WSCALE)
            if b == 5:  # v-agg(0..3) emitted -> superblock 0 ready
                ln1_st = emit_aoproj_sb(0)
            if b == 6:
                # LN1(0) broadcasts emit after kproj(6)/scores(5) so the PE
                # never waits on the rstd scalar chain.
                emit_ln1_finish(0, ln1_st)
            if b == 7:
                hT0 = emit_ffn1(0)

        # tail: sb0's ffn2/LN2 fill the PE while sb1's LN scalar chains run.
        st1 = emit_aoproj_sb(1)
        mr1 = ln_finish_a(st1)
        r20, st20 = emit_ffn2(0, hT0)
        mr20 = ln_finish_a(st20)
        emit_ln1_finish(1, st1, mr=mr1)
        hT1 = emit_ffn1(1)
        emit_ln2_finish(0, r20, st20, mr=mr20)
        r21, st21 = emit_ffn2(1, hT1)
        emit_ln2_finish(1, r21, st21, last=True)

    nc.finalize()
    return nc


def kernel(**inputs):
    # Specialize on actually-zero biases / identity LN affines (checked at
    # runtime; the general program is built when they are nontrivial).
    use_vbias = bool(np.any(np.asarray(inputs["b_v"], dtype=np.float32)))
    affine1 = not (
        np.all(np.asarray(inputs["ln1_g"], dtype=np.float32) == 1.0)
        and not np.any(np.asarray(inputs["ln1_b"], dtype=np.float32))
    )
    affine2 = not (
        np.all(np.asarray(inputs["ln2_g"], dtype=np.float32) == 1.0)
        and not np.any(np.asarray(inputs["ln2_b"], dtype=np.float32))
    )
    use_qbias = bool(np.any(np.asarray(inputs["b_q"], dtype=np.float32)))
    pkey = ("prog", use_vbias, affine1, affine2, use_qbias)
    if pkey not in _CACHE:
        _CACHE[pkey] = build_program(use_vbias, affine1, affine2, use_qbias)
    nc = _CACHE[pkey]

    import ml_dtypes

    f32 = lambda x: np.ascontiguousarray(np.asarray(x), dtype=np.float32)
    bf16 = lambda x: np.ascontiguousarray(np.asarray(x, dtype=np.float32).astype(ml_dtypes.bfloat16))
    fp8 = lambda x: np.ascontiguousarray(
        np.clip(np.asarray(x, dtype=np.float32), -240, 240).astype(ml_dtypes.float8_e4m3))
    query, key_, value = f32(inputs["query"]), f32(inputs["key"]), f32(inputs["value"])

    shared = {}
    packed = np.empty((128, 8 * DT), dtype=np.float32)
    for i, n in enumerate(("b_q", "b_k", "ffn_b1", "ffn_b2",
                           "ln1_g", "ln1_b", "ln2_g", "ln2_b")):
        # b_k is prescaled by WSCALE: the k projection runs against 8*w_k and
        # the x8 is cancelled by the 1/8-valued band mask.
        s = WSCALE if n == "b_k" else 1.0
        packed[:, i * DT : (i + 1) * DT] = (
            s * np.asarray(inputs[n], dtype=np.float32).reshape(DT, 128).T)
    shared["constsP"] = packed
    for n in ("w_v", "ffn_w1", "ffn_w2"):
        shared[n] = bf16(inputs[n])
    for n in ("w_q", "w_k"):
        shared[n] = fp8(np.asarray(inputs[n], dtype=np.float32) * WSCALE)
    shared["bv_row"] = bf16(np.asarray(inputs["b_v"], dtype=np.float32).reshape(1, D))
    # maskT[p, kc, w] = 1/WSCALE where key (kc*128+p) is in window w (the 1/8
    # undoes the x8 k-side weight prescale; exact in bf16)
    p_idx = np.arange(128)[:, None, None]
    kc_idx = np.arange(KC)[None, :, None]
    w_idx = np.arange(128)[None, None, :]
    shared["maskT"] = bf16(
        (w_idx == kc_idx * 32 + p_idx // 4).astype(np.float32) / WSCALE)

    in_maps = []
    for c in range(NCORES):
        bi, half = c // 2, c % 2
        w0 = half * WPC
        m = dict(shared)
        m["qT"] = fp8(query[bi, w0 : w0 + WPC, :].T)
        m["kT"] = fp8(key_[bi, w0 * F : (w0 + WPC) * F, :].T)
        m["vN"] = bf16(value[bi, w0 * F : (w0 + WPC) * F, :])
        in_maps.append(m)

    res = run_bass_kernel_spmd(nc, in_maps, core_ids=list(range(NCORES)))
    _CACHE["last_result"] = res
    out = np.empty((B, SQ, D), dtype=np.float32)
    for c in range(NCORES):
        bi, half = c // 2, c % 2
        w0 = half * WPC
        out[bi, w0 : w0 + WPC, :] = res.results[c]["outT"].T
    return out


# revision 25
# speedup vs baseline: 1.2007x; 1.2007x over previous
"""Trainium2 Bass kernel for nn_AttentionSampling (sparse window attention block).

Sharding: 8 cores, data-parallel, 1024 windows (half a batch) per core; windows are
independent so there is no cross-core communication. q/k live in a transposed
[d, tokens] layout (host pre-transposes) so projections run weight-stationary;
v stays in natural [keys, d] layout so the banded attention aggregation can run
as PE matmuls against the masked score matrix.

Structure (per 128-window / 512-key block):
- k-proj (N=512 bf16 matmuls) -> scores computed directly TRANSPOSED
  ([keys, windows], 16 N=128 matmuls) -> DVE band-mask multiply produces the
  sparse weight matrix W [512 keys, 128 windows] in bf16.
- Attention output via aggregate-then-project: avT = v_nat.T-contracted with W
  (16 N=128 MMs per block); the wv projection + residual add + LN1 stats then
  run once per 512-token superblock at N=512.
- Startup: all engines sit in a fixed ~6.3us rendezvous preamble; DMA issue
  starts ~6.5us and the first 512KB lands ~10.5us. The first q/k projections
  are ki-OUTER over per-d-tile DMA chunks (4 accumulator banks borrowed across
  psum pools) so the PE starts on chunk 0 instead of waiting for whole tiles.
  Warmup matmuls (memset by DVE, which is past the preamble at ~5.8us) bridge
  the issue->land window and open the HAM clock gate.
- DMA issue follows global need order round-robined across the sync/scalar/
  gpsimd queues (aggregate ~330 GB/s; each dma_start costs ~0.75us of NX issue
  time, so scalar-engine issues are spread between its ACT work).
- ffn for superblock 0 is pulled INTO the attention loop (ffn1 at b==7) and the
  post-loop tail interleaves sb0's ffn2/LN2 into the LN1(1) scalar-chain
  latency so the PE never idles waiting on DVE/ACT.
- LN broadcasts are copied PSUM->SBUF 16-bit by ACT so the DVE apply passes run
  at 2x rate; LN2 apply + output DMA are chunked per d-tile across two DMA
  queues to shrink the serial tail.
"""

import sys
import types

try:
    import antenv.axon_hooks  # noqa: F401
except ImportError:
    _m = types.ModuleType("antenv.axon_hooks")
    _m.get_axon_ntff_profile_hook = lambda: None
    _m.set_axon_ntff_profile_hook = lambda h: None
    sys.modules["antenv.axon_hooks"] = _m
    try:
        import antenv

        antenv.axon_hooks = _m
    except ImportError:
        pass

import contextlib

import numpy as np

import concourse.bass as bass
import concourse.bacc as bacc_mod
import concourse.mybir as mybir
import concourse.tile as tile
from concourse.bass import ts, ds
from concourse.bass_utils import run_bass_kernel_spmd

FP32 = mybir.dt.float32
FP16 = mybir.dt.float16
FP8 = mybir.dt.float8e4
AF = mybir.ActivationFunctionType
OP = mybir.AluOpType
DR = mybir.MatmulPerfMode.DoubleRow

MM_DT = mybir.dt.bfloat16  # non-fp8 matmul operands
# q/k and their projection weights are fp8 e4m3 (DoubleRow double-pumped
# matmuls, ~1.4x PE rate, half the DMA bytes). The projection weights are
# scaled x8 host-side so their small values avoid e4m3 subnormals; the k-side
# compensation folds into the band mask (1/8) + 8*b_k, the q-side into the
# drain scale. v/wv/ffn matmuls stay bf16 (fp8 there costs ~4x more error;
# measured final rel err ~1.5e-2 vs the 2e-2 gate). PSUM accumulation is fp32.
# The residual stream and LN stats run in bf16; rstd and apply scratch fp16.
WSCALE = 8.0

B, SQ, SK, D, F = 4, 2048, 8192, 512, 4
NCORES = 8
WPC = B * SQ // NCORES        # 1024 windows (= tokens) per core
KPC = WPC * F                 # 4096 keys per core
NBLK = WPC // 128             # 8 attention blocks: 128 windows / 512 keys
NSB = WPC // 512              # 2 superblocks of 512 tokens
DT = D // 128                 # 4 d-tiles
KC = 4                        # key chunks per block (512 keys / 128)
EPS = 1e-5
N_WARMUP = 10                 # PE warmup matmuls bridging DMA issue->land

_CACHE = {}


def build_program(use_vbias=True, affine1=True, affine2=True, use_qbias=True):
    nc = bacc_mod.Bacc(None, target_bir_lowering=False)

    qT_d = nc.dram_tensor("qT", [D, WPC], FP8, kind="ExternalInput")
    kT_d = nc.dram_tensor("kT", [D, KPC], FP8, kind="ExternalInput")
    vN_d = nc.dram_tensor("vN", [KPC, D], MM_DT, kind="ExternalInput")
    wq_d = nc.dram_tensor("w_q", [D, D], FP8, kind="ExternalInput")
    wk_d = nc.dram_tensor("w_k", [D, D], FP8, kind="ExternalInput")
    wv_d = nc.dram_tensor("w_v", [D, D], MM_DT, kind="ExternalInput")
    w1_d = nc.dram_tensor("ffn_w1", [D, D], MM_DT, kind="ExternalInput")
    w2_d = nc.dram_tensor("ffn_w2", [D, D], MM_DT, kind="ExternalInput")
    # All [D] bias/gain vectors are packed host-side into one [128, 8*DT]
    # tensor (order: b_q, b_k, ffn_b1, ffn_b2, ln1_g, ln1_b, ln2_g, ln2_b).
    consts_d = nc.dram_tensor("constsP", [128, 8 * DT], FP32, kind="ExternalInput")
    bvr_d = nc.dram_tensor("bv_row", [1, D], MM_DT, kind="ExternalInput")
    maskT_d = nc.dram_tensor("maskT", [128, KC, 128], MM_DT, kind="ExternalInput")
    outT_d = nc.dram_tensor("outT", [D, WPC], FP32, kind="ExternalOutput")

    qT_t = qT_d.rearrange("(o p) n -> p o n", p=128)
    kT_t = kT_d.rearrange("(o p) n -> p o n", p=128)
    vN_t = vN_d.rearrange("(nb kc p) d -> p nb kc d", p=128, kc=KC)
    outT_t = outT_d.rearrange("(o p) n -> p o n", p=128)

    with tile.TileContext(nc) as tc, contextlib.ExitStack() as ctx:
        # PSUM budget is 8 banks x 2KB: proj(2) + sc(1) + av(1) + ao(1|2) +
        # stats/bc shared tag(2) [+ srow(1) on the biased path] = 8.
        singles = ctx.enter_context(tc.tile_pool(name="singles", bufs=1))
        qin_p = ctx.enter_context(tc.tile_pool(name="qin", bufs=2))
        kin_p = ctx.enter_context(tc.tile_pool(name="kin", bufs=6))
        vin_p = ctx.enter_context(tc.tile_pool(name="vin", bufs=5))
        ktp_p = ctx.enter_context(tc.tile_pool(name="ktp", bufs=2))
        w_p = ctx.enter_context(tc.tile_pool(name="wsb", bufs=2))
        av_p = ctx.enter_context(tc.tile_pool(name="avsb", bufs=2))
        resid_p = ctx.enter_context(tc.tile_pool(name="resid", bufs=2))
        hT_p = ctx.enter_context(tc.tile_pool(name="hT", bufs=2))
        out_p = ctx.enter_context(tc.tile_pool(name="outp", bufs=2))
        small = ctx.enter_context(tc.tile_pool(name="small", bufs=1))
        ps_proj = ctx.enter_context(tc.tile_pool(name="ps_proj", bufs=2, space="PSUM"))
        ps_sc = ctx.enter_context(tc.tile_pool(name="ps_sc", bufs=1, space="PSUM"))
        ps_av = ctx.enter_context(tc.tile_pool(name="ps_av", bufs=1, space="PSUM"))
        ps_ao = ctx.enter_context(
            tc.tile_pool(name="ps_ao", bufs=1 if use_vbias else 2, space="PSUM"))
        ps_misc = ctx.enter_context(tc.tile_pool(name="ps_misc", bufs=2, space="PSUM"))

        consts_sb = singles.tile([128, 8 * DT], FP32, tag="constsP")
        _CONST_IDX = {"b_q": 0, "b_k": 1, "ffn_b1": 2, "ffn_b2": 3,
                      "ln1_g": 4, "ln1_b": 5, "ln2_g": 6, "ln2_b": 7}

        def load_b(name):
            i = _CONST_IDX[name]
            return consts_sb[:, i * DT : (i + 1) * DT]

        # Warmup scratch: memset on DVE (past the preamble ~5.8us; gpsimd's
        # SWDGE path must not be blocked behind memsets either way).
        warm_sb = singles.tile([128, 512], MM_DT, tag="warm")
        nc.vector.memset(warm_sb, 0.001)
        ones_colb = singles.tile([128, 1], MM_DT, tag="ones_colb")
        nc.vector.memset(ones_colb, 1.0)
        ones_rowb = singles.tile([1, 128], MM_DT, tag="ones_rowb")
        nc.vector.memset(ones_rowb, 1.0)
        ones_rowh = singles.tile([1, 128], FP16, tag="ones_rowh")
        nc.vector.memset(ones_rowh, 1.0)

        # ---- DMA issue: global need order, round-robined across queues ----
        # scalar(A) also runs ACT drains from ~15.5us, so its late loads are
        # spread into the loop; sync(S) has no compute -> all upfront.
        k_tiles = {}
        v_tiles = {}

        def prefetch_k(b, eng):
            k_t = kin_p.tile([128, DT, 512], FP8, tag="k_in", name="k_in")
            eng.dma_start(out=k_t, in_=kT_t[:, :, ts(b, 512)])
            k_tiles[b] = k_t

        def prefetch_v(b, eng):
            v_t = vin_p.tile([128, KC, 512], MM_DT, tag="v_in", name="v_in")
            eng.dma_start(out=v_t, in_=vN_t[:, b, :, :])
            v_tiles[b] = v_t

        def load_w(d, tg, eng, chunks=1, dt=MM_DT):
            t = singles.tile([128, DT, 512], dt, tag=tg)
            src = d.rearrange("(o p) n -> p o n", p=128)
            step = DT // chunks
            for c in range(chunks):
                eng.dma_start(out=t[:, c * step : (c + 1) * step, :],
                              in_=src[:, c * step : (c + 1) * step, :])
            return t

        # First-needed bytes are spread so no queue carries two of
        # {wq+q0, wk+k0} before the PE needs them:
        # A(scalar): consts, wq c0,c1, k2, k3, q1, v2  (+in-loop later)
        # S(sync):   q0 c0,c1, wk c0,c1, v0, k4, wv, k6, v5, w2, v7
        # G(gpsimd): k0, k1, maskT, v1, k5  (+in-loop later)
        nc.scalar.dma_start(out=consts_sb, in_=consts_d[:, :])
        wq_sb = load_w(wq_d, "wq", nc.scalar, chunks=2, dt=FP8)

        q_in = []
        for sb in range(NSB):
            t = qin_p.tile([128, DT, 512], FP8, tag="q_in", name="q_in")
            q_in.append(t)
        for c in range(2):
            nc.sync.dma_start(out=q_in[0][:, 2 * c : 2 * c + 2, :],
                              in_=qT_t[:, 2 * c : 2 * c + 2, ts(0, 512)])

        prefetch_k(0, nc.gpsimd)
        wk_sb = load_w(wk_d, "wk", nc.sync, chunks=2, dt=FP8)
        prefetch_k(1, nc.gpsimd)
        maskT = singles.tile([128, KC, 128], MM_DT, tag="maskT")
        nc.gpsimd.dma_start(out=maskT, in_=maskT_d[:, :, :])
        prefetch_k(2, nc.scalar)
        prefetch_v(0, nc.sync)
        prefetch_v(1, nc.gpsimd)
        prefetch_k(3, nc.scalar)
        prefetch_k(4, nc.sync)
        nc.scalar.dma_start(out=q_in[1], in_=qT_t[:, :, ts(1, 512)])
        prefetch_v(2, nc.scalar)
        prefetch_k(5, nc.gpsimd)
        bq_sb = load_b("b_q")
        bk_sb = load_b("b_k")
        bv_row = None
        if use_vbias:
            bv_row = singles.tile([1, 512], MM_DT, tag="bv_row")
            nc.scalar.dma_start(out=bv_row, in_=bvr_d[:, :])

        late = {}

        # ---- PE warmup: trip the HAM clock gate while DMAs land ----
        for i in range(N_WARMUP):
            wps = ps_proj.tile([128, 512], FP32, tag="proj_ps", name="warm_ps")
            nc.tensor.matmul(wps, lhsT=warm_sb[:, :128], rhs=warm_sb,
                             start=True, stop=True)

        qTp = singles.tile([128, DT, WPC], MM_DT, tag="qTp")
        xT = singles.tile([128, DT, WPC], MM_DT, tag="xT")

        def proj_drain(ps, bias_sb, out_ap, do, relu_dve, scale=1.0):
            if relu_dve:
                if scale == 1.0:
                    nc.vector.tensor_scalar(
                        out=out_ap, in0=ps,
                        scalar1=bias_sb[:, do : do + 1], scalar2=0.0,
                        op0=OP.add, op1=OP.max,
                    )
                else:
                    # bias known zero: fold the fp8 weight prescale out
                    nc.vector.tensor_scalar(
                        out=out_ap, in0=ps, scalar1=scale, scalar2=0.0,
                        op0=OP.mult, op1=OP.max,
                    )
            else:
                nc.scalar.activation(
                    out=out_ap, in_=ps, func=AF.Relu,
                    bias=bias_sb[:, do : do + 1], scale=scale,
                )

        def mm_proj(ps, w_sb, in_sb, do, n, dr):
            if dr:
                for c in range(2):
                    nc.tensor.matmul(
                        ps, lhsT=w_sb[:, 2 * c : 2 * c + 2, ts(do, 128)],
                        rhs=in_sb[:, 2 * c : 2 * c + 2, :n],
                        start=(c == 0), stop=(c == 1), perf_mode=DR,
                    )
            else:
                for ki in range(DT):
                    nc.tensor.matmul(
                        ps, lhsT=w_sb[:, ki, ts(do, 128)], rhs=in_sb[:, ki, :n],
                        start=(ki == 0), stop=(ki == DT - 1),
                    )

        def proj_T(w_sb, bias_sb, in_sb, out_sb, out_col0, n, relu_dve=False,
                   dr=False, scale=1.0):
            for do in range(DT):
                ps = ps_proj.tile([128, 512], FP32, tag="proj_ps", name="proj_ps")
                ps = ps[:, :n]
                mm_proj(ps, w_sb, in_sb, do, n, dr)
                proj_drain(ps, bias_sb, out_sb[:, do, ds(out_col0, n)], do,
                           relu_dve, scale)

        def proj_T_kiouter(w_sb, bias_sb, in_sb, out_sb, out_col0,
                           relu_dve=False, dr=False, scale=1.0):
            # contraction-OUTER accumulation across 4 borrowed psum banks so
            # each MM group depends only on the d-chunk of w/in that has landed.
            pss = [
                ps_proj.tile([128, 512], FP32, tag="proj_ps", name="kio_ps"),
                ps_proj.tile([128, 512], FP32, tag="proj_ps", name="kio_ps"),
                ps_ao.tile([128, 512], FP32, tag="ao_ps", name="kio_ps2"),
                (ps_sc if use_vbias else ps_ao).tile(
                    [128, 512], FP32,
                    tag="sc_ps" if use_vbias else "ao_ps", name="kio_ps3"),
            ]
            if dr:
                for c in range(2):
                    for do in range(DT):
                        nc.tensor.matmul(
                            pss[do], lhsT=w_sb[:, 2 * c : 2 * c + 2, ts(do, 128)],
                            rhs=in_sb[:, 2 * c : 2 * c + 2, :],
                            start=(c == 0), stop=(c == 1), perf_mode=DR,
                        )
            else:
                for ki in range(DT):
                    for do in range(DT):
                        nc.tensor.matmul(
                            pss[do], lhsT=w_sb[:, ki, ts(do, 128)], rhs=in_sb[:, ki, :],
                            start=(ki == 0), stop=(ki == DT - 1),
                        )
            for do in range(DT):
                proj_drain(pss[do], bias_sb, out_sb[:, do, ds(out_col0, 512)],
                           do, relu_dve, scale)

        # ---- phase 1: q projection superblock 0 (chunk-pipelined; sb1 is
        # deferred to b==3 since scores only need it from block 4) ----
        # qTp = relu(q @ (8*wq) / 8 + bq): DVE drain folds the 1/8 when bq==0,
        # else the ACT drain does scale=1/8 + bias.
        proj_T_kiouter(wq_sb, bq_sb, q_in[0], qTp, 0,
                       relu_dve=not use_qbias, dr=True, scale=1.0 / WSCALE)

        # ---- phase 2: attention, software-pipelined ----
        residT = {}  # superblock -> tile [128, DT, 512]
        kTp = {}     # block -> k-projection tile
        W_sb = {}    # block -> masked scoresT (the banded weight matrix)
        av4 = {}     # superblock -> [128, DT, 512] aggregated v (4 blocks)
        sr4 = {}     # superblock -> [1, 512] colsums of W (4 blocks)

        def emit_kproj(b, kiouter=False):
            # kTp8 = relu(k @ (8*wk) + 8*bk) = 8*kTp; the x8 cancels against
            # the 1/8-valued band mask in the score path.
            k_t = k_tiles.pop(b)
            kp = ktp_p.tile([128, DT, 512], MM_DT, tag="kTp", name="kTp")
            if kiouter:
                proj_T_kiouter(wk_sb, bk_sb, k_t, kp, 0, dr=True)
            else:
                proj_T(wk_sb, bk_sb, k_t, kp, 0, 512, dr=True)
            kTp[b] = kp

        def emit_scores(b):
            # scT[k, w] = sum_d kTp[d, k] * qTp[d, w] for this block's keys
            sc_ps = ps_sc.tile([128, KC, 128], FP32, tag="sc_ps", name="sc_ps")
            for kc in range(KC):
                for ki in range(DT):
                    nc.tensor.matmul(
                        sc_ps[:, kc, :],
                        lhsT=kTp[b][:, ki, ts(kc, 128)],
                        rhs=qTp[:, ki, ts(b, 128)],
                        start=(ki == 0), stop=(ki == DT - 1),
                    )
            del kTp[b]
            # band mask -> sparse weight matrix W (bf16, zero off-band)
            w_t = w_p.tile([128, KC, 128], MM_DT, tag="W", name="W")
            nc.vector.tensor_tensor(w_t[:], sc_ps[:], maskT[:], op=OP.mult)
            W_sb[b] = w_t

        def emit_vagg(b):
            sb, col = b // 4, (b % 4) * 128
            v_t = v_tiles.pop(b)
            w_t = W_sb[b]
            av_ps = ps_av.tile([128, DT, 128], FP32, tag="av_ps", name="av_ps")
            for dc in range(DT):
                for kc in range(KC):
                    nc.tensor.matmul(
                        av_ps[:, dc, :],
                        lhsT=v_t[:, kc, ts(dc, 128)],
                        rhs=w_t[:, kc, :],
                        start=(kc == 0), stop=(kc == KC - 1),
                    )
            if use_vbias:
                # srow[w] = sum_k W[k, w]  (for the bias term)
                sr_ps = ps_misc.tile([1, 128], FP32, tag="sr_ps", name="sr_ps", bufs=1)
                for kc in range(KC):
                    nc.tensor.matmul(
                        sr_ps, lhsT=ones_colb, rhs=w_t[:, kc, :],
                        start=(kc == 0), stop=(kc == KC - 1),
                    )
            if col == 0:
                av4[sb] = av_p.tile([128, DT, 512], MM_DT, tag="av4", name="av4")
                if use_vbias:
                    sr4[sb] = small.tile([1, 512], MM_DT, tag="sr4", name="sr4", bufs=2)
            nc.scalar.activation(
                out=av4[sb][:, :, ds(col, 128)], in_=av_ps[:], func=AF.Copy, scale=1.0)
            if use_vbias:
                nc.scalar.activation(
                    out=sr4[sb][:, ds(col, 128)], in_=sr_ps, func=AF.Copy, scale=1.0)
            del W_sb[b]

        def stats_pair():
            """PSUM accumulators for the LN token sums + the squares tile."""
            S1 = ps_misc.tile([1, 512], FP32, tag="st", name="st_sum")
            S2 = ps_misc.tile([1, 512], FP32, tag="st", name="st_sq")
            sqt = hT_p.tile([128, DT, 512], MM_DT, tag="sq", name="sq")
            return S1, S2, sqt

        def emit_stats_dt(st, resid_t, dt):
            """LN stats for one d-tile; interleaved into the producer stream
            one chunk behind so the PE never waits on the DVE square."""
            S1, S2, sqt = st
            nc.vector.tensor_tensor(
                sqt[:, dt, :], resid_t[:, dt, :], resid_t[:, dt, :], op=OP.mult)
            nc.tensor.matmul(S1, lhsT=ones_colb, rhs=resid_t[:, dt, :],
                             start=(dt == 0), stop=(dt == DT - 1))
            nc.tensor.matmul(S2, lhsT=ones_colb, rhs=sqt[:, dt, :],
                             start=(dt == 0), stop=(dt == DT - 1))

        def ln_finish_a(st):
            """LN scalar chain: token sums -> mean (bf16) and rstd (fp16).
            Serial path is 2 DVE ops + 1 ACT Rsqrt (the [1,512] single-lane
            ops cost ~700ns each, so op count dominates the tail):
            mean2 = S1^2/D (direct from PSUM, parallel with the mean copy);
            varD = S2 + D*eps - mean2; rstd = rsqrt(varD/D) = sqrt(D/varD)."""
            S1, S2, _ = st
            mean = small.tile([1, 512], MM_DT, tag="mean", name="mean", bufs=2)
            nc.scalar.activation(out=mean, in_=S1, func=AF.Copy, scale=1.0 / D)
            mean2 = small.tile([1, 512], FP32, tag="m2d", name="mean2")
            nc.vector.scalar_tensor_tensor(
                out=mean2, in0=mean, scalar=float(D), in1=mean,
                op0=OP.mult, op1=OP.mult,
            )
            varD = small.tile([1, 512], FP32, tag="varD", name="varD")
            nc.vector.scalar_tensor_tensor(
                out=varD, in0=S2, scalar=float(D) * EPS, in1=mean2,
                op0=OP.add, op1=OP.subtract,
            )
            r0 = small.tile([1, 512], FP32, tag="r0", name="r0")
            nc.vector.reciprocal_approx_fast(out=r0, in_=varD)
            rstd = small.tile([1, 512], FP16, tag="rstd", name="rstd", bufs=2)
            nc.scalar.activation(out=rstd, in_=r0, func=AF.Sqrt, scale=float(D))
            return mean, rstd

        def emit_ln_finish(st, resid_t, g_sb, gb_sb, out_cb,
                           out_dt_chunked=None, affine=True, mr=None,
                           last=False):
            """Broadcast + apply (+ output) given accumulated token sums."""
            if mr is None:
                mr = ln_finish_a(st)
            mean, rstd = mr

            # bc tiles share the "st" tag/banks; ACT copies them to 16-bit
            # SBUF immediately (frees the banks, enables DVE 2x applies, and
            # lets GpSimd - which has no PSUM port - run half the passes).
            bcm = ps_misc.tile([128, 512], FP32, tag="st", name="bcm")
            nc.tensor.matmul(bcm, lhsT=ones_rowb, rhs=mean, start=True, stop=True)
            bcr = ps_misc.tile([128, 512], FP32, tag="st", name="bcr")
            nc.tensor.matmul(bcr, lhsT=ones_rowh, rhs=rstd, start=True, stop=True)
            bcm_s = small.tile([128, 512], MM_DT, tag="bcm_s", name="bcm_s", bufs=2)
            nc.scalar.activation(out=bcm_s, in_=bcm, func=AF.Copy, scale=1.0)
            if affine or not last:
                bcr_s = small.tile([128, 512], FP16, tag="bcr_s", name="bcr_s", bufs=2)
                nc.scalar.activation(out=bcr_s, in_=bcr, func=AF.Copy, scale=1.0)
            else:
                # final finish: the fp32-out mults get no DVE 2x from a 16-bit
                # bcr anyway, so read PSUM directly and skip the copy latency
                bcr_s = bcr

            # All subs first: they only need bcm_s; the mults drain once
            # bcr_s lands.
            tmp = hT_p.tile([128, DT, 512], FP16, tag="tscr", name="tscr")
            for dt in range(DT):
                nc.vector.tensor_tensor(tmp[:, dt, :], resid_t[:, dt, :], bcm_s, op=OP.subtract)
            for dt in range(DT):
                if affine:
                    nc.vector.tensor_tensor(tmp[:, dt, :], tmp[:, dt, :], bcr_s, op=OP.mult)
                    nc.scalar.activation(
                        out=out_cb(dt), in_=tmp[:, dt, :], func=AF.Identity,
                        bias=gb_sb[:, dt : dt + 1], scale=g_sb[:, dt : dt + 1],
                    )
                else:
                    nc.vector.tensor_tensor(out_cb(dt), tmp[:, dt, :], bcr_s, op=OP.mult)
                if out_dt_chunked:
                    out_dt_chunked(dt)

        def emit_aoproj_sb(sb):
            # ao projection for a whole superblock at N=512, with the residual
            # add and LN1 stats pipelined into the do-chunk stream.
            residT[sb] = resid_p.tile([128, DT, 512], MM_DT, tag="residT", name="residT")
            st = stats_pair()
            for do in range(DT):
                ao_ps = ps_ao.tile([128, 512], FP32, tag="ao_ps", name="ao_ps")
                for ki in range(DT):
                    nc.tensor.matmul(
                        ao_ps, lhsT=late["wv"][:, ki, ts(do, 128)],
                        rhs=av4[sb][:, ki, :],
                        start=(ki == 0), stop=(ki == DT - 1) and not use_vbias,
                    )
                if use_vbias:
                    nc.tensor.matmul(
                        ao_ps, lhsT=bv_row[:, ts(do, 128)], rhs=sr4[sb],
                        start=False, stop=True,
                    )
                nc.vector.tensor_tensor(
                    residT[sb][:, do, :], ao_ps, qTp[:, do, ts(sb, 512)], op=OP.add,
                )
                if do >= 1:
                    emit_stats_dt(st, residT[sb], do - 1)
            emit_stats_dt(st, residT[sb], DT - 1)
            return st

        def emit_ln1_finish(sb, st, mr=None):
            emit_ln_finish(st, residT[sb], late["g1"], late["gb1"],
                           lambda dt: xT[:, dt, ts(sb, 512)], affine=affine1,
                           mr=mr)

        def emit_ffn1(sb):
            hT = hT_p.tile([128, DT, 512], MM_DT, tag="hT", name="hT")
            for ht in range(DT):
                ps = ps_proj.tile([128, 512], FP32, tag="proj_ps", name="ffn1_ps")
                for ki in range(DT):
                    nc.tensor.matmul(
                        ps, lhsT=late["w1"][:, ki, ts(ht, 128)], rhs=xT[:, ki, ts(sb, 512)],
                        start=(ki == 0), stop=(ki == DT - 1),
                    )
                nc.scalar.activation(
                    out=hT[:, ht, :], in_=ps, func=AF.Relu,
                    bias=late["b1"][:, ht : ht + 1], scale=1.0,
                )
            return hT

        def emit_ffn2(sb, hT):
            resid2 = resid_p.tile([128, DT, 512], MM_DT, tag="resid2", name="resid2")
            st = stats_pair()
            for dt in range(DT):
                ps = ps_proj.tile([128, 512], FP32, tag="proj_ps", name="ffn2_ps")
                for hi in range(DT):
                    nc.tensor.matmul(
                        ps, lhsT=late["w2"][:, hi, ts(dt, 128)], rhs=hT[:, hi, :],
                        start=(hi == 0), stop=(hi == DT - 1),
                    )
                nc.vector.scalar_tensor_tensor(
                    out=resid2[:, dt, :], in0=ps, scalar=late["b2"][:, dt : dt + 1],
                    in1=xT[:, dt, ts(sb, 512)], op0=OP.add, op1=OP.add,
                )
                if dt >= 1:
                    emit_stats_dt(st, resid2, dt - 1)
            emit_stats_dt(st, resid2, DT - 1)
            return resid2, st

        def emit_ln2_finish(sb, resid2, st, mr=None, last=False):
            out_sb = out_p.tile([128, DT, 512], FP32, tag="out_sb", name="out_sb")

            def dma_dt(dt):
                (nc.sync if dt % 2 == 0 else nc.scalar).dma_start(
                    out=outT_t[:, dt, ts(sb, 512)], in_=out_sb[:, dt, :]
                )

            emit_ln_finish(st, resid2, late["g2"], late["gb2"],
                           lambda dt: out_sb[:, dt, :], out_dt_chunked=dma_dt,
                           affine=affine2, mr=mr, last=last)

        # pipeline: k-proj(b), scoresT(b-1), v-agg(b-2); ao projection / LN1 /
        # ffn for superblock 0 are interleaved into blocks 5-7.
        hT0 = None
        for b in range(NBLK + 2):
            if b < NBLK:
                emit_kproj(b, kiouter=(b == 0))
            if b == 0:
                late["wv"] = load_w(wv_d, "wv", nc.sync)
                late["g1"] = load_b("ln1_g")
                late["gb1"] = load_b("ln1_b")
                late["b1"] = load_b("ffn_b1")
                late["b2"] = load_b("ffn_b2")
                late["g2"] = load_b("ln2_g")
                late["gb2"] = load_b("ln2_b")
            elif b == 1:
                prefetch_v(3, nc.gpsimd)
                prefetch_k(6, nc.sync)
                prefetch_v(4, nc.scalar)
            elif b == 2:
                prefetch_k(7, nc.scalar)
                late["w1"] = load_w(w1_d, "w1", nc.gpsimd)
            elif b == 3:
                prefetch_v(5, nc.sync)
                late["w2"] = load_w(w2_d, "w2", nc.sync)
            elif b == 4:
                prefetch_v(6, nc.gpsimd)
                prefetch_v(7, nc.sync)
            if 1 <= b <= NBLK:
                emit_scores(b - 1)
            if 2 <= b <= NBLK + 1:
                emit_vagg(b - 2)
            if b == 3:
                # qproj sb1 after vagg(1): its q1/consts DMAs have landed by
                # now and scores(4) (at b==5) is the first consumer.
                proj_T(wq_sb, bq_sb, q_in[1], qTp, 512, 512,
                       relu_dve=not use_qbias, dr=True, scale=1.0 / WSCALE)
            if b == 5:  # v-agg(0..3) emitted -> superblock 0 ready
                ln1_st = emit_aoproj_sb(0)
            if b == 6:
                # LN1(0) broadcasts emit after kproj(6)/scores(5) so the PE
                # never waits on the rstd scalar chain.
                emit_ln1_finish(0, ln1_st)
            if b == 7:
                hT0 = emit_ffn1(0)

        # tail: sb0's ffn2/LN2 fill the PE while sb1's LN scalar chains run.
        st1 = emit_aoproj_sb(1)
        mr1 = ln_finish_a(st1)
        r20, st20 = emit_ffn2(0, hT0)
        mr20 = ln_finish_a(st20)
        emit_ln1_finish(1, st1, mr=mr1)
        hT1 = emit_ffn1(1)
        emit_ln2_finish(0, r20, st20, mr=mr20)
        r21, st21 = emit_ffn2(1, hT1)
        emit_ln2_finish(1, r21, st21, last=True)

    nc.finalize()
    return nc


def kernel(**inputs):
    # Specialize on actually-zero biases / identity LN affines (checked at
    # runtime; the general program is built when they are nontrivial).
    use_vbias = bool(np.any(np.asarray(inputs["b_v"], dtype=np.float32)))
    affine1 = not (
        np.all(np.asarray(inputs["ln1_g"], dtype=np.float32) == 1.0)
        and not np.any(np.asarray(inputs["ln1_b"], dtype=np.float32))
    )
    affine2 = not (
        np.all(np.asarray(inputs["ln2_g"], dtype=np.float32) == 1.0)
        and not np.any(np.asarray(inputs["ln2_b"], dtype=np.float32))
    )
    use_qbias = bool(np.any(np.asarray(inputs["b_q"], dtype=np.float32)))
    pkey = ("prog", use_vbias, affine1, affine2, use_qbias)
    if pkey not in _CACHE:
        _CACHE[pkey] = build_program(use_vbias, affine1, affine2, use_qbias)
    nc = _CACHE[pkey]

    import ml_dtypes

    f32 = lambda x: np.ascontiguousarray(np.asarray(x), dtype=np.float32)
    bf16 = lambda x: np.ascontiguousarray(np.asarray(x, dtype=np.float32).astype(ml_dtypes.bfloat16))
    fp8 = lambda x: np.ascontiguousarray(
        np.clip(np.asarray(x, dtype=np.float32), -240, 240).astype(ml_dtypes.float8_e4m3))
    query, key_, value = f32(inputs["query"]), f32(inputs["key"]), f32(inputs["value"])

    shared = {}
    packed = np.empty((128, 8 * DT), dtype=np.float32)
    for i, n in enumerate(("b_q", "b_k", "ffn_b1", "ffn_b2",
                           "ln1_g", "ln1_b", "ln2_g", "ln2_b")):
        # b_k is prescaled by WSCALE: the k projection runs against 8*w_k and
        # the x8 is cancelled by the 1/8-valued band mask.
        s = WSCALE if n == "b_k" else 1.0
        packed[:, i * DT : (i + 1) * DT] = (
            s * np.asarray(inputs[n], dtype=np.float32).reshape(DT, 128).T)
    shared["constsP"] = packed
    for n in ("w_v", "ffn_w1", "ffn_w2"):
        shared[n] = bf16(inputs[n])
    for n in ("w_q", "w_k"):
        shared[n] = fp8(np.asarray(inputs[n], dtype=np.float32) * WSCALE)
    shared["bv_row"] = bf16(np.asarray(inputs["b_v"], dtype=np.float32).reshape(1, D))
    # maskT[p, kc, w] = 1/WSCALE where key (kc*128+p) is in window w (the 1/8
    # undoes the x8 k-side weight prescale; exact in bf16)
    p_idx = np.arange(128)[:, None, None]
    kc_idx = np.arange(KC)[None, :, None]
    w_idx = np.arange(128)[None, None, :]
    shared["maskT"] = bf16(
        (w_idx == kc_idx * 32 + p_idx // 4).astype(np.float32) / WSCALE)

    in_maps = []
    for c in range(NCORES):
        bi, half = c // 2, c % 2
        w0 = half * WPC
        m = dict(shared)
        m["qT"] = fp8(query[bi, w0 : w0 + WPC, :].T)
        m["kT"] = fp8(key_[bi, w0 * F : (w0 + WPC) * F, :].T)
        m["vN"] = bf16(value[bi, w0 * F : (w0 + WPC) * F, :])
        in_maps.append(m)

    res = run_bass_kernel_spmd(nc, in_maps, core_ids=list(range(NCORES)))
    _CACHE["last_result"] = res
    out = np.empty((B, SQ, D), dtype=np.float32)
    for c in range(NCORES):
        bi, half = c // 2, c % 2
        w0 = half * WPC
        out[bi, w0 : w0 + WPC, :] = res.results[c]["outT"].T
    return out


# revision 30
# speedup vs baseline: 1.2022x; 1.0012x over previous
"""Trainium2 Bass kernel for nn_AttentionSampling (sparse window attention block).

Sharding: 8 cores, data-parallel, 1024 windows (half a batch) per core; windows are
independent so there is no cross-core communication. q/k live in a transposed
[d, tokens] layout (host pre-transposes) so projections run weight-stationary;
v stays in natural [keys, d] layout so the banded attention aggregation can run
as PE matmuls against the masked score matrix.

Structure (per 128-window / 512-key block):
- k-proj (N=512 bf16 matmuls) -> scores computed directly TRANSPOSED
  ([keys, windows], 16 N=128 matmuls) -> DVE band-mask multiply produces the
  sparse weight matrix W [512 keys, 128 windows] in bf16.
- Attention output via aggregate-then-project: avT = v_nat.T-contracted with W
  (16 N=128 MMs per block); the wv projection + residual add + LN1 stats then
  run once per 512-token superblock at N=512.
- Startup: all engines sit in a fixed ~6.3us rendezvous preamble; DMA issue
  starts ~6.5us and the first 512KB lands ~10.5us. The first q/k projections
  are ki-OUTER over per-d-tile DMA chunks (4 accumulator banks borrowed across
  psum pools) so the PE starts on chunk 0 instead of waiting for whole tiles.
  Warmup matmuls (memset by DVE, which is past the preamble at ~5.8us) bridge
  the issue->land window and open the HAM clock gate.
- DMA issue follows global need order round-robined across the sync/scalar/
  gpsimd queues (aggregate ~330 GB/s; each dma_start costs ~0.75us of NX issue
  time, so scalar-engine issues are spread between its ACT work).
- ffn for superblock 0 is pulled INTO the attention loop (ffn1 at b==7) and the
  post-loop tail interleaves sb0's ffn2/LN2 into the LN1(1) scalar-chain
  latency so the PE never idles waiting on DVE/ACT.
- LN broadcasts are copied PSUM->SBUF 16-bit by ACT so the DVE apply passes run
  at 2x rate; LN2 apply + output DMA are chunked per d-tile across two DMA
  queues to shrink the serial tail.
"""

import sys
import types

try:
    import antenv.axon_hooks  # noqa: F401
except ImportError:
    _m = types.ModuleType("antenv.axon_hooks")
    _m.get_axon_ntff_profile_hook = lambda: None
    _m.set_axon_ntff_profile_hook = lambda h: None
    sys.modules["antenv.axon_hooks"] = _m
    try:
        import antenv

        antenv.axon_hooks = _m
    except ImportError:
        pass

import contextlib

import numpy as np

import concourse.bass as bass
import concourse.bacc as bacc_mod
import concourse.mybir as mybir
import concourse.tile as tile
from concourse.bass import ts, ds
from concourse.bass_utils import run_bass_kernel_spmd

FP32 = mybir.dt.float32
FP16 = mybir.dt.float16
FP8 = mybir.dt.float8e4
AF = mybir.ActivationFunctionType
OP = mybir.AluOpType
DR = mybir.MatmulPerfMode.DoubleRow

MM_DT = mybir.dt.bfloat16  # non-fp8 matmul operands
# q/k and their projection weights are fp8 e4m3 (DoubleRow double-pumped
# matmuls, ~1.4x PE rate, half the DMA bytes). The projection weights are
# scaled x8 host-side so their small values avoid e4m3 subnormals; the k-side
# compensation folds into the band mask (1/8) + 8*b_k, the q-side into the
# drain scale. v/wv/ffn matmuls stay bf16 (fp8 there costs ~4x more error;
# measured final rel err ~1.5e-2 vs the 2e-2 gate). PSUM accumulation is fp32.
# The residual stream and LN stats run in bf16; rstd and apply scratch fp16.
WSCALE = 8.0

B, SQ, SK, D, F = 4, 2048, 8192, 512, 4
NCORES = 8
WPC = B * SQ // NCORES        # 1024 windows (= tokens) per core
KPC = WPC * F                 # 4096 keys per core
NBLK = WPC // 128             # 8 attention blocks: 128 windows / 512 keys
NSB = WPC // 512              # 2 superblocks of 512 tokens
DT = D // 128                 # 4 d-tiles
KC = 4                        # key chunks per block (512 keys / 128)
EPS = 1e-5
N_WARMUP = 14                 # PE warmup matmuls bridging DMA issue->land

_CACHE = {}


def build_program(use_vbias=True, affine1=True, affine2=True, use_qbias=True):
    nc = bacc_mod.Bacc(None, target_bir_lowering=False)

    qT_d = nc.dram_tensor("qT", [D, WPC], FP8, kind="ExternalInput")
    kT_d = nc.dram_tensor("kT", [D, KPC], FP8, kind="ExternalInput")
    vN_d = nc.dram_tensor("vN", [KPC, D], MM_DT, kind="ExternalInput")
    wq_d = nc.dram_tensor("w_q", [D, D], FP8, kind="ExternalInput")
    wk_d = nc.dram_tensor("w_k", [D, D], FP8, kind="ExternalInput")
    wv_d = nc.dram_tensor("w_v", [D, D], MM_DT, kind="ExternalInput")
    w1_d = nc.dram_tensor("ffn_w1", [D, D], MM_DT, kind="ExternalInput")
    w2_d = nc.dram_tensor("ffn_w2", [D, D], MM_DT, kind="ExternalInput")
    # All [D] bias/gain vectors are packed host-side into one [128, 8*DT]
    # tensor (order: b_q, b_k, ffn_b1, ffn_b2, ln1_g, ln1_b, ln2_g, ln2_b).
    consts_d = nc.dram_tensor("constsP", [128, 8 * DT], FP32, kind="ExternalInput")
    bvr_d = nc.dram_tensor("bv_row", [1, D], MM_DT, kind="ExternalInput")
    maskT_d = nc.dram_tensor("maskT", [128, KC, 128], MM_DT, kind="ExternalInput")
    # bf16 output: halves the output DMA and enables 2x DVE apply passes; the
    # host upcasts to fp32 (costs <1e-3 of the 2e-2 rel-err budget).
    outT_d = nc.dram_tensor("outT", [D, WPC], MM_DT, kind="ExternalOutput")

    qT_t = qT_d.rearrange("(o p) n -> p o n", p=128)
    kT_t = kT_d.rearrange("(o p) n -> p o n", p=128)
    vN_t = vN_d.rearrange("(nb kc p) d -> p nb kc d", p=128, kc=KC)
    outT_t = outT_d.rearrange("(o p) n -> p o n", p=128)

    with tile.TileContext(nc) as tc, contextlib.ExitStack() as ctx:
        # PSUM budget is 8 banks x 2KB: proj(2) + sc(1) + av(1) + ao(1|2) +
        # stats/bc shared tag(2) [+ srow(1) on the biased path] = 8.
        singles = ctx.enter_context(tc.tile_pool(name="singles", bufs=1))
        qin_p = ctx.enter_context(tc.tile_pool(name="qin", bufs=2))
        kin_p = ctx.enter_context(tc.tile_pool(name="kin", bufs=6))
        vin_p = ctx.enter_context(tc.tile_pool(name="vin", bufs=5))
        ktp_p = ctx.enter_context(tc.tile_pool(name="ktp", bufs=2))
        w_p = ctx.enter_context(tc.tile_pool(name="wsb", bufs=2))
        av_p = ctx.enter_context(tc.tile_pool(name="avsb", bufs=2))
        resid_p = ctx.enter_context(tc.tile_pool(name="resid", bufs=2))
        hT_p = ctx.enter_context(tc.tile_pool(name="hT", bufs=2))
        out_p = ctx.enter_context(tc.tile_pool(name="outp", bufs=2))
        small = ctx.enter_context(tc.tile_pool(name="small", bufs=1))
        ps_proj = ctx.enter_context(tc.tile_pool(name="ps_proj", bufs=2, space="PSUM"))
        ps_sc = ctx.enter_context(tc.tile_pool(name="ps_sc", bufs=1, space="PSUM"))
        ps_av = ctx.enter_context(tc.tile_pool(name="ps_av", bufs=1, space="PSUM"))
        ps_ao = ctx.enter_context(
            tc.tile_pool(name="ps_ao", bufs=1 if use_vbias else 2, space="PSUM"))
        ps_misc = ctx.enter_context(tc.tile_pool(name="ps_misc", bufs=2, space="PSUM"))

        consts_sb = singles.tile([128, 8 * DT], FP32, tag="constsP")
        _CONST_IDX = {"b_q": 0, "b_k": 1, "ffn_b1": 2, "ffn_b2": 3,
                      "ln1_g": 4, "ln1_b": 5, "ln2_g": 6, "ln2_b": 7}

        def load_b(name):
            i = _CONST_IDX[name]
            return consts_sb[:, i * DT : (i + 1) * DT]

        # Warmup scratch: memset on DVE (past the preamble ~5.8us; gpsimd's
        # SWDGE path must not be blocked behind memsets either way).
        warm_sb = singles.tile([128, 512], MM_DT, tag="warm")
        nc.vector.memset(warm_sb, 0.001)
        ones_colb = singles.tile([128, 1], MM_DT, tag="ones_colb")
        nc.vector.memset(ones_colb, 1.0)
        ones_rowb = singles.tile([1, 128], MM_DT, tag="ones_rowb")
        nc.vector.memset(ones_rowb, 1.0)
        ones_rowh = singles.tile([1, 128], FP16, tag="ones_rowh")
        nc.vector.memset(ones_rowh, 1.0)

        # ---- DMA issue: global need order, round-robined across queues ----
        # scalar(A) also runs ACT drains from ~15.5us, so its late loads are
        # spread into the loop; sync(S) has no compute -> all upfront.
        k_tiles = {}
        v_tiles = {}

        def prefetch_k(b, eng):
            k_t = kin_p.tile([128, DT, 512], FP8, tag="k_in", name="k_in")
            eng.dma_start(out=k_t, in_=kT_t[:, :, ts(b, 512)])
            k_tiles[b] = k_t

        def prefetch_v(b, eng):
            v_t = vin_p.tile([128, KC, 512], MM_DT, tag="v_in", name="v_in")
            eng.dma_start(out=v_t, in_=vN_t[:, b, :, :])
            v_tiles[b] = v_t

        def load_w(d, tg, eng, chunks=1, dt=MM_DT):
            t = singles.tile([128, DT, 512], dt, tag=tg)
            src = d.rearrange("(o p) n -> p o n", p=128)
            step = DT // chunks
            for c in range(chunks):
                eng.dma_start(out=t[:, c * step : (c + 1) * step, :],
                              in_=src[:, c * step : (c + 1) * step, :])
            return t

        # First-needed bytes are spread so no queue carries two of
        # {wq+q0, wk+k0} before the PE needs them. The SWDGE (gpsimd) queue
        # measures ~3x slower than the HWDGE rings early on, so it only gets
        # items with slack:
        # A(scalar): consts, wq c0,c1, k1, k3, q1, v2  (+in-loop later)
        # S(sync):   q0 c0,c1, wk c0,c1, v0, k2, k4, wv, k6, v5, w2, v7
        # G(gpsimd): k0, maskT, v1, k5  (+in-loop later)
        nc.scalar.dma_start(out=consts_sb, in_=consts_d[:, :])
        wq_sb = load_w(wq_d, "wq", nc.scalar, chunks=2, dt=FP8)

        q_in = []
        for sb in range(NSB):
            t = qin_p.tile([128, DT, 512], FP8, tag="q_in", name="q_in")
            q_in.append(t)
        for c in range(2):
            nc.sync.dma_start(out=q_in[0][:, 2 * c : 2 * c + 2, :],
                              in_=qT_t[:, 2 * c : 2 * c + 2, ts(0, 512)])

        prefetch_k(0, nc.gpsimd)
        wk_sb = load_w(wk_d, "wk", nc.sync, chunks=2, dt=FP8)
        prefetch_k(1, nc.scalar)
        maskT = singles.tile([128, KC, 128], MM_DT, tag="maskT")
        nc.gpsimd.dma_start(out=maskT, in_=maskT_d[:, :, :])
        prefetch_v(0, nc.sync)
        prefetch_k(2, nc.sync)
        prefetch_v(1, nc.gpsimd)
        prefetch_k(3, nc.scalar)
        prefetch_k(4, nc.sync)
        nc.scalar.dma_start(out=q_in[1], in_=qT_t[:, :, ts(1, 512)])
        prefetch_v(2, nc.scalar)
        prefetch_k(5, nc.gpsimd)
        bq_sb = load_b("b_q")
        bk_sb = load_b("b_k")
        bv_row = None
        if use_vbias:
            bv_row = singles.tile([1, 512], MM_DT, tag="bv_row")
            nc.scalar.dma_start(out=bv_row, in_=bvr_d[:, :])

        late = {}

        # ---- PE warmup: trip the HAM clock gate while DMAs land ----
        for i in range(N_WARMUP):
            wps = ps_proj.tile([128, 512], FP32, tag="proj_ps", name="warm_ps")
            nc.tensor.matmul(wps, lhsT=warm_sb[:, :128], rhs=warm_sb,
                             start=True, stop=True)

        qTp = singles.tile([128, DT, WPC], MM_DT, tag="qTp")
        xT = singles.tile([128, DT, WPC], MM_DT, tag="xT")

        def proj_drain(ps, bias_sb, out_ap, do, relu_dve, scale=1.0):
            if relu_dve:
                if scale == 1.0:
                    nc.vector.tensor_scalar(
                        out=out_ap, in0=ps,
                        scalar1=bias_sb[:, do : do + 1], scalar2=0.0,
                        op0=OP.add, op1=OP.max,
                    )
                else:
                    # bias known zero: fold the fp8 weight prescale out
                    nc.vector.tensor_scalar(
                        out=out_ap, in0=ps, scalar1=scale, scalar2=0.0,
                        op0=OP.mult, op1=OP.max,
                    )
            else:
                nc.scalar.activation(
                    out=out_ap, in_=ps, func=AF.Relu,
                    bias=bias_sb[:, do : do + 1], scale=scale,
                )

        def mm_proj(ps, w_sb, in_sb, do, n, dr):
            if dr:
                for c in range(2):
                    nc.tensor.matmul(
                        ps, lhsT=w_sb[:, 2 * c : 2 * c + 2, ts(do, 128)],
                        rhs=in_sb[:, 2 * c : 2 * c + 2, :n],
                        start=(c == 0), stop=(c == 1), perf_mode=DR,
                    )
            else:
                for ki in range(DT):
                    nc.tensor.matmul(
                        ps, lhsT=w_sb[:, ki, ts(do, 128)], rhs=in_sb[:, ki, :n],
                        start=(ki == 0), stop=(ki == DT - 1),
                    )

        def proj_T(w_sb, bias_sb, in_sb, out_sb, out_col0, n, relu_dve=False,
                   dr=False, scale=1.0):
            for do in range(DT):
                ps = ps_proj.tile([128, 512], FP32, tag="proj_ps", name="proj_ps")
                ps = ps[:, :n]
                mm_proj(ps, w_sb, in_sb, do, n, dr)
                proj_drain(ps, bias_sb, out_sb[:, do, ds(out_col0, n)], do,
                           relu_dve, scale)

        def proj_T_kiouter(w_sb, bias_sb, in_sb, out_sb, out_col0,
                           relu_dve=False, dr=False, scale=1.0):
            # contraction-OUTER accumulation across 4 borrowed psum banks so
            # each MM group depends only on the d-chunk of w/in that has landed.
            pss = [
                ps_proj.tile([128, 512], FP32, tag="proj_ps", name="kio_ps"),
                ps_proj.tile([128, 512], FP32, tag="proj_ps", name="kio_ps"),
                ps_ao.tile([128, 512], FP32, tag="ao_ps", name="kio_ps2"),
                (ps_sc if use_vbias else ps_ao).tile(
                    [128, 512], FP32,
                    tag="sc_ps" if use_vbias else "ao_ps", name="kio_ps3"),
            ]
            if dr:
                for c in range(2):
                    for do in range(DT):
                        nc.tensor.matmul(
                            pss[do], lhsT=w_sb[:, 2 * c : 2 * c + 2, ts(do, 128)],
                            rhs=in_sb[:, 2 * c : 2 * c + 2, :],
                            start=(c == 0), stop=(c == 1), perf_mode=DR,
                        )
            else:
                for ki in range(DT):
                    for do in range(DT):
                        nc.tensor.matmul(
                            pss[do], lhsT=w_sb[:, ki, ts(do, 128)], rhs=in_sb[:, ki, :],
                            start=(ki == 0), stop=(ki == DT - 1),
                        )
            for do in range(DT):
                proj_drain(pss[do], bias_sb, out_sb[:, do, ds(out_col0, 512)],
                           do, relu_dve, scale)

        # ---- phase 1: q projection superblock 0 (chunk-pipelined; sb1 is
        # deferred to b==3 since scores only need it from block 4) ----
        # qTp = relu(q @ (8*wq) / 8 + bq): DVE drain folds the 1/8 when bq==0,
        # else the ACT drain does scale=1/8 + bias.
        proj_T_kiouter(wq_sb, bq_sb, q_in[0], qTp, 0,
                       relu_dve=not use_qbias, dr=True, scale=1.0 / WSCALE)

        # ---- phase 2: attention, software-pipelined ----
        residT = {}  # superblock -> tile [128, DT, 512]
        kTp = {}     # block -> k-projection tile
        W_sb = {}    # block -> masked scoresT (the banded weight matrix)
        av4 = {}     # superblock -> [128, DT, 512] aggregated v (4 blocks)
        sr4 = {}     # superblock -> [1, 512] colsums of W (4 blocks)

        def emit_kproj(b, kiouter=False):
            # kTp8 = relu(k @ (8*wk) + 8*bk) = 8*kTp; the x8 cancels against
            # the 1/8-valued band mask in the score path.
            k_t = k_tiles.pop(b)
            kp = ktp_p.tile([128, DT, 512], MM_DT, tag="kTp", name="kTp")
            if kiouter:
                proj_T_kiouter(wk_sb, bk_sb, k_t, kp, 0, dr=True)
            else:
                proj_T(wk_sb, bk_sb, k_t, kp, 0, 512, dr=True)
            kTp[b] = kp

        def emit_scores(b):
            # scT[k, w] = sum_d kTp[d, k] * qTp[d, w] for this block's keys
            sc_ps = ps_sc.tile([128, KC, 128], FP32, tag="sc_ps", name="sc_ps")
            for kc in range(KC):
                for ki in range(DT):
                    nc.tensor.matmul(
                        sc_ps[:, kc, :],
                        lhsT=kTp[b][:, ki, ts(kc, 128)],
                        rhs=qTp[:, ki, ts(b, 128)],
                        start=(ki == 0), stop=(ki == DT - 1),
                    )
            del kTp[b]
            # band mask -> sparse weight matrix W (bf16, zero off-band)
            w_t = w_p.tile([128, KC, 128], MM_DT, tag="W", name="W")
            nc.vector.tensor_tensor(w_t[:], sc_ps[:], maskT[:], op=OP.mult)
            W_sb[b] = w_t

        def emit_vagg(b):
            sb, col = b // 4, (b % 4) * 128
            v_t = v_tiles.pop(b)
            w_t = W_sb[b]
            av_ps = ps_av.tile([128, DT, 128], FP32, tag="av_ps", name="av_ps")
            for dc in range(DT):
                for kc in range(KC):
                    nc.tensor.matmul(
                        av_ps[:, dc, :],
                        lhsT=v_t[:, kc, ts(dc, 128)],
                        rhs=w_t[:, kc, :],
                        start=(kc == 0), stop=(kc == KC - 1),
                    )
            if use_vbias:
                # srow[w] = sum_k W[k, w]  (for the bias term)
                sr_ps = ps_misc.tile([1, 128], FP32, tag="sr_ps", name="sr_ps", bufs=1)
                for kc in range(KC):
                    nc.tensor.matmul(
                        sr_ps, lhsT=ones_colb, rhs=w_t[:, kc, :],
                        start=(kc == 0), stop=(kc == KC - 1),
                    )
            if col == 0:
                av4[sb] = av_p.tile([128, DT, 512], MM_DT, tag="av4", name="av4")
                if use_vbias:
                    sr4[sb] = small.tile([1, 512], MM_DT, tag="sr4", name="sr4", bufs=2)
            nc.scalar.activation(
                out=av4[sb][:, :, ds(col, 128)], in_=av_ps[:], func=AF.Copy, scale=1.0)
            if use_vbias:
                nc.scalar.activation(
                    out=sr4[sb][:, ds(col, 128)], in_=sr_ps, func=AF.Copy, scale=1.0)
            del W_sb[b]

        def stats_pair():
            """PSUM accumulators for the LN token sums + the squares tile."""
            S1 = ps_misc.tile([1, 512], FP32, tag="st", name="st_sum")
            S2 = ps_misc.tile([1, 512], FP32, tag="st", name="st_sq")
            sqt = hT_p.tile([128, DT, 512], MM_DT, tag="sq", name="sq")
            return S1, S2, sqt

        def emit_stats_dt(st, resid_t, dt):
            """LN stats for one d-tile; interleaved into the producer stream
            one chunk behind so the PE never waits on the DVE square."""
            S1, S2, sqt = st
            nc.vector.tensor_tensor(
                sqt[:, dt, :], resid_t[:, dt, :], resid_t[:, dt, :], op=OP.mult)
            nc.tensor.matmul(S1, lhsT=ones_colb, rhs=resid_t[:, dt, :],
                             start=(dt == 0), stop=(dt == DT - 1))
            nc.tensor.matmul(S2, lhsT=ones_colb, rhs=sqt[:, dt, :],
                             start=(dt == 0), stop=(dt == DT - 1))

        def ln_finish_a(st):
            """LN scalar chain: token sums -> mean (bf16) and rstd (fp16).
            Serial path is 2 DVE ops + 1 ACT Rsqrt (the [1,512] single-lane
            ops cost ~700ns each, so op count dominates the tail):
            mean2 = S1^2/D (direct from PSUM, parallel with the mean copy);
            varD = S2 + D*eps - mean2; rstd = rsqrt(varD/D) = sqrt(D/varD)."""
            S1, S2, _ = st
            mean = small.tile([1, 512], MM_DT, tag="mean", name="mean", bufs=2)
            nc.scalar.activation(out=mean, in_=S1, func=AF.Copy, scale=1.0 / D)
            mean2 = small.tile([1, 512], FP32, tag="m2d", name="mean2")
            nc.vector.scalar_tensor_tensor(
                out=mean2, in0=mean, scalar=float(D), in1=mean,
                op0=OP.mult, op1=OP.mult,
            )
            varD = small.tile([1, 512], FP32, tag="varD", name="varD")
            nc.vector.scalar_tensor_tensor(
                out=varD, in0=S2, scalar=float(D) * EPS, in1=mean2,
                op0=OP.add, op1=OP.subtract,
            )
            r0 = small.tile([1, 512], FP32, tag="r0", name="r0")
            nc.vector.reciprocal_approx_fast(out=r0, in_=varD)
            rstd = small.tile([1, 512], FP16, tag="rstd", name="rstd", bufs=2)
            nc.scalar.activation(out=rstd, in_=r0, func=AF.Sqrt, scale=float(D))
            return mean, rstd

        def emit_ln_finish(st, resid_t, g_sb, gb_sb, out_cb,
                           out_dt_chunked=None, affine=True, mr=None,
                           last=False):
            """Broadcast + apply (+ output) given accumulated token sums."""
            if mr is None:
                mr = ln_finish_a(st)
            mean, rstd = mr

            # bc tiles share the "st" tag/banks; ACT copies them to 16-bit
            # SBUF immediately (frees the banks, enables DVE 2x applies, and
            # lets GpSimd - which has no PSUM port - run half the passes).
            bcm = ps_misc.tile([128, 512], FP32, tag="st", name="bcm")
            nc.tensor.matmul(bcm, lhsT=ones_rowb, rhs=mean, start=True, stop=True)
            bcr = ps_misc.tile([128, 512], FP32, tag="st", name="bcr")
            nc.tensor.matmul(bcr, lhsT=ones_rowh, rhs=rstd, start=True, stop=True)
            bcm_s = small.tile([128, 512], MM_DT, tag="bcm_s", name="bcm_s", bufs=2)
            nc.scalar.activation(out=bcm_s, in_=bcm, func=AF.Copy, scale=1.0)
            if affine or not last:
                bcr_s = small.tile([128, 512], FP16, tag="bcr_s", name="bcr_s", bufs=2)
                nc.scalar.activation(out=bcr_s, in_=bcr, func=AF.Copy, scale=1.0)
            else:
                # final finish: the fp32-out mults get no DVE 2x from a 16-bit
                # bcr anyway, so read PSUM directly and skip the copy latency
                bcr_s = bcr

            # All subs first: they only need bcm_s; the mults drain once
            # bcr_s lands.
            tmp = hT_p.tile([128, DT, 512], FP16, tag="tscr", name="tscr")
            for dt in range(DT):
                nc.vector.tensor_tensor(tmp[:, dt, :], resid_t[:, dt, :], bcm_s, op=OP.subtract)
            for dt in range(DT):
                if affine:
                    nc.vector.tensor_tensor(tmp[:, dt, :], tmp[:, dt, :], bcr_s, op=OP.mult)
                    nc.scalar.activation(
                        out=out_cb(dt), in_=tmp[:, dt, :], func=AF.Identity,
                        bias=gb_sb[:, dt : dt + 1], scale=g_sb[:, dt : dt + 1],
                    )
                else:
                    nc.vector.tensor_tensor(out_cb(dt), tmp[:, dt, :], bcr_s, op=OP.mult)
                if out_dt_chunked:
                    out_dt_chunked(dt)

        def emit_aoproj_sb(sb):
            # ao projection for a whole superblock at N=512, with the residual
            # add and LN1 stats pipelined into the do-chunk stream.
            residT[sb] = resid_p.tile([128, DT, 512], MM_DT, tag="residT", name="residT")
            st = stats_pair()
            for do in range(DT):
                ao_ps = ps_ao.tile([128, 512], FP32, tag="ao_ps", name="ao_ps")
                for ki in range(DT):
                    nc.tensor.matmul(
                        ao_ps, lhsT=late["wv"][:, ki, ts(do, 128)],
                        rhs=av4[sb][:, ki, :],
                        start=(ki == 0), stop=(ki == DT - 1) and not use_vbias,
                    )
                if use_vbias:
                    nc.tensor.matmul(
                        ao_ps, lhsT=bv_row[:, ts(do, 128)], rhs=sr4[sb],
                        start=False, stop=True,
                    )
                nc.vector.tensor_tensor(
                    residT[sb][:, do, :], ao_ps, qTp[:, do, ts(sb, 512)], op=OP.add,
                )
                if do >= 1:
                    emit_stats_dt(st, residT[sb], do - 1)
            emit_stats_dt(st, residT[sb], DT - 1)
            return st

        def emit_ln1_finish(sb, st, mr=None):
            emit_ln_finish(st, residT[sb], late["g1"], late["gb1"],
                           lambda dt: xT[:, dt, ts(sb, 512)], affine=affine1,
                           mr=mr)

        def emit_ffn1(sb):
            hT = hT_p.tile([128, DT, 512], MM_DT, tag="hT", name="hT")
            for ht in range(DT):
                ps = ps_proj.tile([128, 512], FP32, tag="proj_ps", name="ffn1_ps")
                for ki in range(DT):
                    nc.tensor.matmul(
                        ps, lhsT=late["w1"][:, ki, ts(ht, 128)], rhs=xT[:, ki, ts(sb, 512)],
                        start=(ki == 0), stop=(ki == DT - 1),
                    )
                nc.scalar.activation(
                    out=hT[:, ht, :], in_=ps, func=AF.Relu,
                    bias=late["b1"][:, ht : ht + 1], scale=1.0,
                )
            return hT

        def emit_ffn2(sb, hT):
            resid2 = resid_p.tile([128, DT, 512], MM_DT, tag="resid2", name="resid2")
            st = stats_pair()
            for dt in range(DT):
                ps = ps_proj.tile([128, 512], FP32, tag="proj_ps", name="ffn2_ps")
                for hi in range(DT):
                    nc.tensor.matmul(
                        ps, lhsT=late["w2"][:, hi, ts(dt, 128)], rhs=hT[:, hi, :],
                        start=(hi == 0), stop=(hi == DT - 1),
                    )
                nc.vector.scalar_tensor_tensor(
                    out=resid2[:, dt, :], in0=ps, scalar=late["b2"][:, dt : dt + 1],
                    in1=xT[:, dt, ts(sb, 512)], op0=OP.add, op1=OP.add,
                )
                if dt >= 1:
                    emit_stats_dt(st, resid2, dt - 1)
            emit_stats_dt(st, resid2, DT - 1)
            return resid2, st

        def emit_ln2_finish(sb, resid2, st, mr=None, last=False):
            out_sb = out_p.tile([128, DT, 512], MM_DT, tag="out_sb", name="out_sb")

            def dma_dt(dt):
                (nc.sync if dt % 2 == 0 else nc.scalar).dma_start(
                    out=outT_t[:, dt, ts(sb, 512)], in_=out_sb[:, dt, :]
                )

            emit_ln_finish(st, resid2, late["g2"], late["gb2"],
                           lambda dt: out_sb[:, dt, :], out_dt_chunked=dma_dt,
                           affine=affine2, mr=mr, last=last)

        # pipeline: k-proj(b), scoresT(b-1), v-agg(b-2); ao projection / LN1 /
        # ffn for superblock 0 are interleaved into blocks 5-7.
        hT0 = None
        for b in range(NBLK + 2):
            if b < NBLK:
                emit_kproj(b, kiouter=(b == 0))
            if b == 0:
                late["wv"] = load_w(wv_d, "wv", nc.sync)
                late["g1"] = load_b("ln1_g")
                late["gb1"] = load_b("ln1_b")
                late["b1"] = load_b("ffn_b1")
                late["b2"] = load_b("ffn_b2")
                late["g2"] = load_b("ln2_g")
                late["gb2"] = load_b("ln2_b")
            elif b == 1:
                prefetch_v(3, nc.gpsimd)
                prefetch_k(6, nc.sync)
                prefetch_v(4, nc.scalar)
            elif b == 2:
                prefetch_k(7, nc.scalar)
                late["w1"] = load_w(w1_d, "w1", nc.gpsimd)
            elif b == 3:
                prefetch_v(5, nc.sync)
                late["w2"] = load_w(w2_d, "w2", nc.sync)
            elif b == 4:
                prefetch_v(6, nc.gpsimd)
                prefetch_v(7, nc.sync)
            if 1 <= b <= NBLK:
                emit_scores(b - 1)
            if 2 <= b <= NBLK + 1:
                emit_vagg(b - 2)
            if b == 3:
                # qproj sb1 after vagg(1): its q1/consts DMAs have landed by
                # now and scores(4) (at b==5) is the first consumer.
                proj_T(wq_sb, bq_sb, q_in[1], qTp, 512, 512,
                       relu_dve=not use_qbias, dr=True, scale=1.0 / WSCALE)
            if b == 5:  # v-agg(0..3) emitted -> superblock 0 ready
                ln1_st = emit_aoproj_sb(0)
            if b == 6:
                # LN1(0) broadcasts emit after kproj(6)/scores(5) so the PE
                # never waits on the rstd scalar chain.
                emit_ln1_finish(0, ln1_st)
            if b == 7:
                hT0 = emit_ffn1(0)

        # tail: sb0's ffn2/LN2 fill the PE while sb1's LN scalar chains run.
        st1 = emit_aoproj_sb(1)
        mr1 = ln_finish_a(st1)
        r20, st20 = emit_ffn2(0, hT0)
        mr20 = ln_finish_a(st20)
        emit_ln1_finish(1, st1, mr=mr1)
        hT1 = emit_ffn1(1)
        emit_ln2_finish(0, r20, st20, mr=mr20)
        r21, st21 = emit_ffn2(1, hT1)
        emit_ln2_finish(1, r21, st21, last=True)

    nc.finalize()
    return nc


def kernel(**inputs):
    # Specialize on actually-zero biases / identity LN affines (checked at
    # runtime; the general program is built when they are nontrivial).
    use_vbias = bool(np.any(np.asarray(inputs["b_v"], dtype=np.float32)))
    affine1 = not (
        np.all(np.asarray(inputs["ln1_g"], dtype=np.float32) == 1.0)
        and not np.any(np.asarray(inputs["ln1_b"], dtype=np.float32))
    )
    affine2 = not (
        np.all(np.asarray(inputs["ln2_g"], dtype=np.float32) == 1.0)
        and not np.any(np.asarray(inputs["ln2_b"], dtype=np.float32))
    )
    use_qbias = bool(np.any(np.asarray(inputs["b_q"], dtype=np.float32)))
    pkey = ("prog", use_vbias, affine1, affine2, use_qbias)
    if pkey not in _CACHE:
        _CACHE[pkey] = build_program(use_vbias, affine1, affine2, use_qbias)
    nc = _CACHE[pkey]

    import ml_dtypes

    f32 = lambda x: np.ascontiguousarray(np.asarray(x), dtype=np.float32)
    bf16 = lambda x: np.ascontiguousarray(np.asarray(x, dtype=np.float32).astype(ml_dtypes.bfloat16))
    fp8 = lambda x: np.ascontiguousarray(
        np.clip(np.asarray(x, dtype=np.float32), -240, 240).astype(ml_dtypes.float8_e4m3))
    query, key_, value = f32(inputs["query"]), f32(inputs["key"]), f32(inputs["value"])

    shared = {}
    packed = np.empty((128, 8 * DT), dtype=np.float32)
    for i, n in enumerate(("b_q", "b_k", "ffn_b1", "ffn_b2",
                           "ln1_g", "ln1_b", "ln2_g", "ln2_b")):
        # b_k is prescaled by WSCALE: the k projection runs against 8*w_k and
        # the x8 is cancelled by the 1/8-valued band mask.
        s = WSCALE if n == "b_k" else 1.0
        packed[:, i * DT : (i + 1) * DT] = (
            s * np.asarray(inputs[n], dtype=np.float32).reshape(DT, 128).T)
    shared["constsP"] = packed
    for n in ("w_v", "ffn_w1", "ffn_w2"):
        shared[n] = bf16(inputs[n])
    for n in ("w_q", "w_k"):
        shared[n] = fp8(np.asarray(inputs[n], dtype=np.float32) * WSCALE)
    shared["bv_row"] = bf16(np.asarray(inputs["b_v"], dtype=np.float32).reshape(1, D))
    # maskT[p, kc, w] = 1/WSCALE where key (kc*128+p) is in window w (the 1/8
    # undoes the x8 k-side weight prescale; exact in bf16)
    p_idx = np.arange(128)[:, None, None]
    kc_idx = np.arange(KC)[None, :, None]
    w_idx = np.arange(128)[None, None, :]
    shared["maskT"] = bf16(
        (w_idx == kc_idx * 32 + p_idx // 4).astype(np.float32) / WSCALE)

    in_maps = []
    for c in range(NCORES):
        bi, half = c // 2, c % 2
        w0 = half * WPC
        m = dict(shared)
        m["qT"] = fp8(query[bi, w0 : w0 + WPC, :].T)
        m["kT"] = fp8(key_[bi, w0 * F : (w0 + WPC) * F, :].T)
        m["vN"] = bf16(value[bi, w0 * F : (w0 + WPC) * F, :])
        in_maps.append(m)

    res = run_bass_kernel_spmd(nc, in_maps, core_ids=list(range(NCORES)))
    _CACHE["last_result"] = res
    out = np.empty((B, SQ, D), dtype=np.float32)
    for c in range(NCORES):
        bi, half = c // 2, c % 2
        w0 = half * WPC
        out[bi, w0 : w0 + WPC, :] = np.asarray(
            res.results[c]["outT"], dtype=np.float32).T
    return out
